# revision 1
# baseline (speedup 1.0000x reference)
"""Trainium2 Bass kernel for nn_DAGRN; fully on-device 8-core SPMD."""
import sys
sys.path.insert(0, "/opt/trn_rl_repo")
import numpy as np


N = 50000
E = 800000
F = 64
H = 256
ID = 32
DYN = 16
B = 256
T = 128
STEPS = 3
W = 8
P = 6250          # nodes per core
PP = 6272         # padded (49*128)
NTILE = 49
NFULL = W * PP    # 50176
HALF = NFULL // 2  # 25088
BL = B // W       # 32 sequences per core


# ---------------------------------------------------------------- host prep

def prep_edges(edge_src, edge_dst):
    """Bucket edges by dst core, group by (dst_tile, src_half), pad each
    group to x128 with a structure shared across cores.
    Returns (NTg [49,2] int, esrc16 [W,16,EPAD//16] i16,
             dstrel [W,128,EPAD//128] f32)."""
    src = np.asarray(edge_src, np.int64)
    dst = np.asarray(edge_dst, np.int64)
    core = dst // P
    ldst = (dst - core * P).astype(np.int32)
    srcpad = ((src // P) * PP + src % P).astype(np.int32)
    half = (srcpad >= HALF).astype(np.int32)
    src16 = (srcpad - half * HALF).astype(np.int16)
    tile = ldst >> 7
    rel = (ldst & 127).astype(np.float32)

    key = ((core * NTILE + tile) * 2 + half).astype(np.int64)
    order = np.argsort(key, kind="stable")
    ks = key[order]
    counts = np.bincount(key, minlength=W * NTILE * 2).reshape(W, NTILE, 2)
    NTg = (np.ceil(counts / 128).astype(np.int64)).max(axis=0)  # [49,2]
    NTg = np.maximum(NTg, 1)  # avoid empty groups (psum never written)
    EPAD = int(NTg.sum()) * 128

    goff = np.zeros(NTILE * 2 + 1, np.int64)
    goff[1:] = np.cumsum(NTg.reshape(-1)) * 128  # group offsets in padded list

    # rank within group for each sorted edge
    gstart = np.zeros(W * NTILE * 2, np.int64)
    uniq, first = np.unique(ks, return_index=True)
    gstart[uniq] = first
    rank = np.arange(E) - gstart[ks]
    gl = ks - (ks // (NTILE * 2)) * (NTILE * 2)  # local group id (tile*2+half)
    dest = goff[gl] + rank
    co = ks // (NTILE * 2)

    esrc16 = np.zeros((W, EPAD), np.int16)
    dstrel = np.full((W, EPAD), -1.0, np.float32)
    esrc16[co, dest] = src16[order]
    dstrel[co, dest] = rel[order]

    esrc_w = np.ascontiguousarray(
        esrc16.reshape(W, EPAD // 16, 16).transpose(0, 2, 1))
    dstrel_w = np.ascontiguousarray(
        dstrel.reshape(W, EPAD // 128, 128).transpose(0, 2, 1))
    return NTg, esrc_w, dstrel_w


def wrap16(idx):
    """int16 [n] -> [16, n//16] wrapped layout."""
    idx = np.asarray(idx, np.int16)
    return np.ascontiguousarray(idx.reshape(-1, 16).T)


def prep_host(inp):
    """Build per-core in_maps + compile structure from full inputs."""
    f32 = np.float32
    g = lambda k: np.asarray(inp[k], f32)
    traj = np.asarray(inp["traj"], np.int64)
    lengths = np.asarray(inp["lengths"], np.int64)
    NTg, esrc_w, dstrel_w = prep_edges(inp["edge_src"], inp["edge_dst"])

    x0 = g("x0")
    dyn = g("dyn_feat")
    id_emb = g("id_emb")

    # x0t_aug per core: [65, PP] (row 64 = ones)
    x0p = np.zeros((W, PP, F), f32)
    x0p[:, :P] = x0.reshape(W, P, F)
    x0t = np.concatenate(
        [x0p.transpose(0, 2, 1), np.ones((W, 1, PP), f32)], 1)

    # GGNN weights
    Wz, Wr, Wn = g("Wz"), g("Wr"), g("Wn")
    bz, br, bn = g("bz"), g("br"), g("bn")
    wx_aug = np.concatenate([
        np.concatenate([Wz[:F], bz[None]], 0),
        np.concatenate([Wr[:F], br[None]], 0),
        np.concatenate([Wn[:F], bn[None]], 0)], 1)      # [65, 768]
    whc = np.concatenate([Wz[F:], Wr[F:], Wn[F:]], 1)   # [256, 768]
    wproj_aug = np.concatenate([g("Wproj"), g("bproj")[None]], 0)  # [65,256]

    # FiLM
    film = np.tanh((dyn @ g("film_W1") + g("film_b1")) @ g("film_W2")
                   + g("film_b2"))                       # [B, 512]
    gp1 = 1.0 + film[:, :H]
    bf = film[:, H:]
    gp1T = np.ascontiguousarray(
        gp1.reshape(W, BL, 2, 128).transpose(0, 3, 2, 1))  # [W,128,2,BL]
    bfT = np.ascontiguousarray(
        bf.reshape(W, BL, 2, 128).transpose(0, 3, 2, 1))

    # traj / mask / reversal
    trajpad = ((traj // P) * PP + traj % P).astype(np.int32)   # [B, T]
    tt = np.arange(T)
    mask = (tt[None, :] < lengths[:, None])                    # [B, T]
    rev = np.maximum(lengths[:, None] - 1 - tt[None, :], 0).astype(np.int64)
    trajidx = np.ascontiguousarray(
        trajpad.reshape(W, BL, T).transpose(0, 2, 1))          # [W,128,BL]

    maskrow = np.ascontiguousarray(
        mask.astype(f32).reshape(W, BL, T).transpose(0, 2, 1)
        .reshape(W, 1, T * BL))                                # col t*BL+b
    attn_b2 = float(np.asarray(inp["attn_b2"]).reshape(-1)[0])
    valid = (traj != 0)
    negb = np.where(valid, attn_b2, -1e9).astype(f32)
    negrow = np.ascontiguousarray(
        negb.reshape(W, BL, T).transpose(0, 2, 1).reshape(W, 1, T * BL))

    # bwd xg gather idx (pairs over b): per step s, idx [96]:
    # j' = m*16 + b', src pair = rev[b,s]*96 + m*16 + b'
    revl = rev.reshape(W, BL, T)                               # [W, b, s]
    m_ = np.arange(6)
    bp = np.arange(BL // 2)
    # [W, s, m, b'] -> value rev[w,b=2b'? pairs are (2b',2b'+1) same b? NO:
    # pair index b' covers b=2b',2b'+1; both have same rev only if rev same
    # for both b -> NOT same. Pairs must be along contiguous axis with
    # identical src mapping. xg col = t*192 + m*32 + b; pair (2j,2j+1) =
    # b=2b',2b'+1 of same (t,m). src = rev[b,s]*192+m*32+b depends on b ->
    # pair src differs between the two b -> CANNOT pair over b.
    # Instead gather at d=2 over... fall back: idx per element invalid.
    # Solution: reorder xg col layout to (t, m, b) with b inner BUT gather
    # along t only: src pair for (m, b even/odd) has different rev.
    # => use layout col = t*192 + b*6 + m  (m inner!). Then pair (2j,2j+1)
    # = m=2m',2m'+1 of same (t,b): src = rev[b,s]*96 + b*3 + m'  OK!
    apgx = (revl[:, None, :, :] * 96                       # [W,1,b,s] ->
            ).transpose(0, 3, 2, 1)                        # [W,s,b,1]
    apgx = apgx + (np.arange(BL) * 3)[None, None, :, None] \
        + m_[:3][None, None, None, :]                      # [W,s,b,3]
    apgx = apgx.reshape(W, T, BL * 3).astype(np.int16)     # [W, s, 96]
    # wrapped: [W, 16, T*6]: idx j of step s -> [j%16, s*6 + j//16]
    apgx_w = np.ascontiguousarray(
        apgx.reshape(W, T, 6, 16).transpose(0, 3, 1, 2)
        .reshape(W, 16, T * 6))

    # outs un-reverse idx (pairs over c? outs col = t*64 + c*32 + b,
    # pair=(b even,odd) same problem -> use col = t*64 + b*2 + c (c inner):
    # src pair = rev[b,t]*32 + b  (c'=pair of c=0,1, d=2)
    apgo = (revl * 32 + np.arange(BL)[None, :, None]) \
        .transpose(0, 2, 1).reshape(W, T * BL).astype(np.int16)  # [W,(t,b)]
    apgo_w = np.ascontiguousarray(
        apgo.reshape(W, T * BL // 16, 16).transpose(0, 2, 1))    # [W,16,256]

    # iddyn [49, BL*T] col = b*T + t
    idt = id_emb[traj]                                     # [B, T, 32]
    iddyn = np.zeros((W, 49, BL * T), f32)
    iddyn[:, :ID] = idt.reshape(W, BL, T, ID).transpose(0, 3, 1, 2) \
        .reshape(W, ID, BL * T)
    iddyn[:, ID:ID + DYN] = np.repeat(
        dyn.reshape(W, BL, DYN).transpose(0, 2, 1), T, axis=2
    ).reshape(W, DYN, BL * T)
    iddyn[:, ID + DYN] = 1.0

    # GRU weights (bf16), bh folded into bi
    bf16 = np.dtype("bfloat16") if hasattr(np, "bfloat16") else None

    def to_bf16(x):
        import ml_dtypes
        return np.asarray(x, np.float32).astype(ml_dtypes.bfloat16)

    def wi_chunks(Wi, bi, bh):
        Wi = np.asarray(Wi, f32)
        bt = (np.asarray(bi, f32) + np.asarray(bh, f32))[None]  # [1,768]
        k2 = np.concatenate([Wi[256:304], bt], 0)               # [49, 768]
        return to_bf16(Wi[0:128]), to_bf16(Wi[128:256]), to_bf16(k2)

    wif = wi_chunks(inp["gru_Wi_f"], inp["gru_bi_f"], inp["gru_bh_f"])
    wib = wi_chunks(inp["gru_Wi_b"], inp["gru_bi_b"], inp["gru_bh_b"])
    whf = to_bf16(np.asarray(inp["gru_Wh_f"], f32))   # [256, 768]
    whb = to_bf16(np.asarray(inp["gru_Wh_b"], f32))

    # attention W1 chunks [out_f 2, out_b 2, dyn+bias 17]
    aW1 = g("attn_W1")
    a_k = [to_bf16(aW1[i * 128:(i + 1) * 128]) for i in range(4)] \
        + [to_bf16(np.concatenate([aW1[512:528], g("attn_b1")[None]], 0))]
    aW2 = to_bf16(g("attn_W2"))                       # [256, 1]

    lnT = np.ascontiguousarray(g("ln_g").reshape(4, 128).T)   # [128,4]
    lnbT = np.ascontiguousarray(g("ln_b").reshape(4, 128).T)
    dynb = np.concatenate(
        [dyn.reshape(W, BL, DYN).transpose(0, 2, 1),
         np.ones((W, 1, BL), f32)], 1)                # [W, 17, BL]

    gW = g("gate_W")                                  # [1040, 512]
    g_k = [to_bf16(gW[i * 128:(i + 1) * 128]) for i in range(8)] \
        + [to_bf16(np.concatenate([gW[1024:1040], g("gate_b")[None]], 0))]
    fW1 = g("fc_W1")                                  # [528, 256]
    f_k = [to_bf16(fW1[i * 128:(i + 1) * 128]) for i in range(4)] \
        + [to_bf16(np.concatenate([fW1[512:528], g("fc_b1")[None]], 0))]
    fW2 = to_bf16(g("fc_W2"))                         # [256, 1]
    fc_b2 = float(np.asarray(inp["fc_b2"]).reshape(-1)[0])

    in_maps = []
    for c in range(W):
        m = dict(
            x0t=x0t[c], esrc=esrc_w[c], dstrel=dstrel_w[c],
            wx_aug=wx_aug, whc=whc, wproj_aug=wproj_aug,
            gp1T=gp1T[c], bfT=bfT[c], trajidx=trajidx[c],
            maskrow=maskrow[c], negrow=negrow[c],
            apgx=apgx_w[c], apgo=apgo_w[c], iddyn=to_bf16(iddyn[c]),
            wif0=wif[0], wif1=wif[1], wif2=wif[2],
            wib0=wib[0], wib1=wib[1], wib2=wib[2],
            whf=whf, whb=whb,
            aw0=a_k[0], aw1=a_k[1], aw2=a_k[2], aw3=a_k[3], aw4=a_k[4],
            aW2=aW2, lnT=lnT, lnbT=lnbT, dynb=to_bf16(dynb[c]),
            gw0=g_k[0], gw1=g_k[1], gw2=g_k[2], gw3=g_k[3], gw4=g_k[4],
            gw5=g_k[5], gw6=g_k[6], gw7=g_k[7], gw8=g_k[8],
            fw0=f_k[0], fw1=f_k[1], fw2=f_k[2], fw3=f_k[3], fw4=f_k[4],
            fW2=fW2,
        )
        in_maps.append(m)
    return NTg, in_maps, fc_b2


# ---------------------------------------------------------------- device program

def build_program(NTg, debug_h=False, seq=True, steps=STEPS, gelu_ok=True,
                  dbg=False):
    import concourse.bacc as bacc
    import concourse.mybir as mybir
    import concourse.tile as tile
    import concourse.bass as bass
    from concourse.masks import make_identity

    f32 = mybir.dt.float32
    bf16 = mybir.dt.bfloat16
    i16 = mybir.dt.int16
    i32 = mybir.dt.int32
    AF = mybir.ActivationFunctionType
    OP = mybir.AluOpType
    NTg = np.asarray(NTg)
    ETILES = int(NTg.sum())
    EPAD = ETILES * 128
    MAXNT = int(NTg.max())

    nc = bacc.Bacc("TRN2", target_bir_lowering=False, debug=False,
                   num_devices=W)
    DT = lambda *a, **k: nc.dram_tensor(*a, **k)
    # --- inputs
    x0t = DT("x0t", [F + 1, PP], f32, kind="ExternalInput")
    esrc = DT("esrc", [16, EPAD // 16], i16, kind="ExternalInput")
    dstrel = DT("dstrel", [128, ETILES], f32, kind="ExternalInput")
    wx_aug = DT("wx_aug", [F + 1, 3 * H], f32, kind="ExternalInput")
    whc = DT("whc", [H, 3 * H], f32, kind="ExternalInput")
    wproj_aug = DT("wproj_aug", [F + 1, H], f32, kind="ExternalInput")
    gp1T = DT("gp1T", [128, 2, BL], f32, kind="ExternalInput")
    bfT = DT("bfT", [128, 2, BL], f32, kind="ExternalInput")
    trajidx = DT("trajidx", [128, BL], i32, kind="ExternalInput")
    maskrow = DT("maskrow", [1, T * BL], f32, kind="ExternalInput")
    negrow = DT("negrow", [1, T * BL], f32, kind="ExternalInput")
    apgx = DT("apgx", [16, T * 6], i16, kind="ExternalInput")
    apgo = DT("apgo", [16, T * BL // 16], i16, kind="ExternalInput")
    iddyn = DT("iddyn", [F // 64 * 49, BL * T], bf16, kind="ExternalInput")
    wif = [DT(f"wif{i}", [128, 3 * H] if i < 2 else [49, 3 * H], bf16,
              kind="ExternalInput") for i in range(3)]
    wib = [DT(f"wib{i}", [128, 3 * H] if i < 2 else [49, 3 * H], bf16,
              kind="ExternalInput") for i in range(3)]
    whf = DT("whf", [H, 3 * H], bf16, kind="ExternalInput")
    whb = DT("whb", [H, 3 * H], bf16, kind="ExternalInput")
    aw = [DT(f"aw{i}", [128 if i < 4 else 17, H], bf16,
             kind="ExternalInput") for i in range(5)]
    aW2 = DT("aW2", [H, 1], bf16, kind="ExternalInput")
    lnT = DT("lnT", [128, 4], f32, kind="ExternalInput")
    lnbT = DT("lnbT", [128, 4], f32, kind="ExternalInput")
    dynb = DT("dynb", [DYN + 1, BL], bf16, kind="ExternalInput")
    gw = [DT(f"gw{i}", [128 if i < 8 else 17, 2 * H], bf16,
             kind="ExternalInput") for i in range(9)]
    fw = [DT(f"fw{i}", [128 if i < 4 else 17, H], bf16,
             kind="ExternalInput") for i in range(5)]
    fW2 = DT("fW2", [H, 1], bf16, kind="ExternalInput")
    # --- outputs
    out32 = DT("out32", [1, BL], f32, kind="ExternalOutput")
    dbgs = {}
    if dbg:
        for nm, shp in [("dxf", [128, 256]), ("dxg", [128, 192]),
                        ("dxgb", [128, 192]), ("dof", [128, 256]),
                        ("dorb", [128, 256]), ("dob", [128, 256]),
                        ("ds1", [128, 512]), ("dscb", [BL, T]),
                        ("dal", [BL, T]), ("dctx", [128, 128]),
                        ("drnl", [128, 128]), ("dzf", [128, 128]),
                        ("dhfc", [128, 64]), ("dhtf", [128, 64]),
                        ("dhtb", [128, 64]), ("dxs", [128, 256]),
                        ("diddyn", [49, 128])]:
            dbgs[nm] = DT(nm, shp, f32, kind="ExternalOutput")
    if debug_h:
        hdbg = DT("hdbg", [PP, H], f32, kind="ExternalOutput")
    # --- internal dram
    hfull = DT("hfull", [NFULL, H], f32, kind="Internal", addr_space="Shared")

    with tile.TileContext(nc) as tc:
        with (tc.tile_pool(name="cst", bufs=1) as cst,
              tc.tile_pool(name="dram", bufs=1, space="DRAM") as dramp):
            cstg_cm = tc.tile_pool(name="cstg", bufs=1)
            cstg = cstg_cm.__enter__()
            hshard = dramp.tile([PP, H], f32)
            # constants
            ident = cst.tile([128, 128], f32)
            make_identity(nc, ident[:])
            iotai = cst.tile([128, 128], i32)
            nc.gpsimd.iota(iotai[:], pattern=[[1, 128]], base=0,
                           channel_multiplier=0)
            iotaf = cst.tile([128, 128], f32)
            nc.vector.tensor_copy(out=iotaf[:], in_=iotai[:])
            onesrow = cst.tile([1, 128], f32)
            nc.vector.memset(onesrow[:], 1.0)
            onescol = cst.tile([128, 1], f32)
            nc.vector.memset(onescol[:], 1.0)

            x0t_sb = cstg.tile([F + 1, PP], f32)
            nc.sync.dma_start(out=x0t_sb[:], in_=x0t[:])
            wx_sb = cstg.tile([F + 1, 3 * H], f32)
            nc.sync.dma_start(out=wx_sb[:], in_=wx_aug[:])
            whc_sb = [cstg.tile([128, 3 * H], f32, tag=f"whc{k}",
                                name=f"whc{k}") for k in range(2)]
            for k in range(2):
                nc.sync.dma_start(out=whc_sb[k][:],
                                  in_=whc[k * 128:(k + 1) * 128, :])
            wproj_sb = cstg.tile([F + 1, H], f32)
            nc.sync.dma_start(out=wproj_sb[:], in_=wproj_aug[:])
            esrc_sb = cstg.tile([128, EPAD // 16], i16)
            for g8 in range(8):
                nc.sync.dma_start(out=esrc_sb[g8 * 16:(g8 + 1) * 16, :],
                                  in_=esrc[:])
            dstrel_sb = cstg.tile([128, ETILES], f32)
            nc.sync.dma_start(out=dstrel_sb[:], in_=dstrel[:])

            # ---------------- projection: h0 = tanh(x0 @ Wproj + b)
            with (
                tc.tile_pool(name="pj", bufs=3, space="PSUM") as pj,
                tc.tile_pool(name="sprj", bufs=3) as sprj,
            ):
                for t in range(NTILE):
                    ps = pj.tile([128, H], f32, tag="pj")
                    nc.tensor.matmul(
                        out=ps[:], lhsT=x0t_sb[:, t * 128:(t + 1) * 128],
                        rhs=wproj_sb[:], start=True, stop=True)
                    h0 = sprj.tile([128, H], f32, tag="h0")
                    nc.scalar.activation(h0[:], ps[:], AF.Tanh)
                    nc.sync.dma_start(out=hshard[t * 128:(t + 1) * 128, :],
                                      in_=h0[:])
            nc.gpsimd.collective_compute(
                "AllGather", mybir.AluOpType.bypass,
                replica_groups=[list(range(W))],
                ins=[hshard[:]], outs=[hfull[:]])

            # ---------------- GGNN steps
            for s in range(steps):
                with (
                    tc.tile_pool(name=f"agp{s}", bufs=2, space="PSUM") as agp,
                    tc.tile_pool(name=f"gtp{s}", bufs=3, space="PSUM") as gtp,
                    tc.tile_pool(name=f"trp{s}", bufs=2, space="PSUM") as trp,
                    tc.tile_pool(name=f"sg{s}", bufs=3) as sg,
                    tc.tile_pool(name=f"sd{s}", bufs=2) as sd,
                ):
                    off16 = 0
                    offt = 0
                    for t in range(NTILE):
                        agg_ps = agp.tile([128, H], f32, tag="agg")
                        ntl, nth = int(NTg[t][0]), int(NTg[t][1])
                        first = True
                        for hx, nt in ((0, ntl), (1, nth)):
                            if nt == 0:
                                continue
                            msgs = sg.tile([128, MAXNT, H], f32, tag="msgs")
                            src_half = (hfull[0:HALF, :] if hx == 0
                                        else hfull[HALF:NFULL, :])
                            # dma_gather dies above 1024 idxs per instr
                            for g0 in range(0, nt, 8):
                                gnt = min(8, nt - g0)
                                nc.gpsimd.dma_gather(
                                    out_ap=msgs[:, g0:g0 + gnt, :],
                                    in_ap=src_half,
                                    idxs_ap=esrc_sb[
                                        :, off16 + g0 * 8:
                                        off16 + (g0 + gnt) * 8],
                                    num_idxs=gnt * 128,
                                    num_idxs_reg=gnt * 128,
                                    elem_size=H)
                            sstr = sg.tile([128, MAXNT, 128], f32, tag="sstr")
                            nc.vector.tensor_tensor(
                                out=sstr[:, 0:nt, :],
                                in0=dstrel_sb[:, offt:offt + nt]
                                .unsqueeze(2).to_broadcast([128, nt, 128]),
                                in1=iotaf[:].unsqueeze(1)
                                .to_broadcast([128, nt, 128]),
                                op=OP.is_equal)
                            for i in range(nt):
                                last = (hx == 1 or nth == 0) and i == nt - 1
                                nc.tensor.matmul(
                                    out=agg_ps[:], lhsT=sstr[:, i, :],
                                    rhs=msgs[:, i, :],
                                    start=first, stop=last)
                                first = False
                            off16 += nt * 8
                            offt += nt
                        # dense update for this node tile
                        agg = sd.tile([128, H], f32, tag="agg_sb")
                        nc.vector.tensor_copy(out=agg[:], in_=agg_ps[:])
                        aggT = sd.tile([128, 2, 128], f32, tag="aggT")
                        for c in range(2):
                            tp = trp.tile([128, 128], f32, tag="tr")
                            nc.tensor.transpose(
                                out=tp[:], in_=agg[:, c * 128:(c + 1) * 128],
                                identity=ident[:])
                            nc.vector.tensor_copy(out=aggT[:, c, :], in_=tp[:])
                        xsl = x0t_sb[:, t * 128:(t + 1) * 128]
                        zps = gtp.tile([128, H], f32, tag="gate")
                        nc.tensor.matmul(out=zps[:], lhsT=xsl,
                                         rhs=wx_sb[:, 0:H], start=True,
                                         stop=False)
                        for c in range(2):
                            nc.tensor.matmul(
                                out=zps[:], lhsT=aggT[:, c, :],
                                rhs=whc_sb[c][:, 0:H], start=False,
                                stop=(c == 1))
                        zg = sd.tile([128, H], f32, tag="zg")
                        nc.scalar.activation(zg[:], zps[:], AF.Sigmoid)
                        rps = gtp.tile([128, H], f32, tag="gate")
                        nc.tensor.matmul(out=rps[:], lhsT=xsl,
                                         rhs=wx_sb[:, H:2 * H], start=True,
                                         stop=False)
                        for c in range(2):
                            nc.tensor.matmul(
                                out=rps[:], lhsT=aggT[:, c, :],
                                rhs=whc_sb[c][:, H:2 * H], start=False,
                                stop=(c == 1))
                        rg = sd.tile([128, H], f32, tag="rg")
                        nc.scalar.activation(rg[:], rps[:], AF.Sigmoid)
                        rh = sd.tile([128, H], f32, tag="rh")
                        nc.vector.tensor_tensor(out=rh[:], in0=rg[:],
                                                in1=agg[:], op=OP.mult)
                        rhT = sd.tile([128, 2, 128], f32, tag="rhT")
                        for c in range(2):
                            tp = trp.tile([128, 128], f32, tag="tr")
                            nc.tensor.transpose(
                                out=tp[:], in_=rh[:, c * 128:(c + 1) * 128],
                                identity=ident[:])
                            nc.vector.tensor_copy(out=rhT[:, c, :], in_=tp[:])
                        nps = gtp.tile([128, H], f32, tag="gate")
                        nc.tensor.matmul(out=nps[:], lhsT=xsl,
                                         rhs=wx_sb[:, 2 * H:3 * H],
                                         start=True, stop=False)
                        for c in range(2):
                            nc.tensor.matmul(
                                out=nps[:], lhsT=rhT[:, c, :],
                                rhs=whc_sb[c][:, 2 * H:3 * H], start=False,
                                stop=(c == 1))
                        ht = sd.tile([128, H], f32, tag="ht")
                        nc.scalar.activation(ht[:], nps[:], AF.Tanh)
                        d1 = sd.tile([128, H], f32, tag="d1")
                        nc.vector.tensor_tensor(out=d1[:], in0=ht[:],
                                                in1=agg[:], op=OP.subtract)
                        d2 = sd.tile([128, H], f32, tag="d2")
                        nc.vector.tensor_tensor(out=d2[:], in0=zg[:],
                                                in1=d1[:], op=OP.mult)
                        hnew = sd.tile([128, H], f32, tag="hnew")
                        nc.vector.tensor_tensor(out=hnew[:], in0=agg[:],
                                                in1=d2[:], op=OP.add)
                        nc.sync.dma_start(
                            out=hshard[t * 128:(t + 1) * 128, :],
                            in_=hnew[:])
                nc.gpsimd.collective_compute(
                    "AllGather", mybir.AluOpType.bypass,
                    replica_groups=[list(range(W))],
                    ins=[hshard[:]], outs=[hfull[:]])

            if debug_h:
                with tc.tile_pool(name="dbg", bufs=2) as dbp:
                    for t in range(NTILE):
                        tb = dbp.tile([128, H], f32, tag="dbg")
                        nc.sync.dma_start(
                            out=tb[:], in_=hshard[t * 128:(t + 1) * 128, :])
                        nc.sync.dma_start(
                            out=hdbg[t * 128:(t + 1) * 128, :], in_=tb[:])
            cstg_cm.__exit__(None, None, None)
            if seq:
                _seq_phase(nc, tc, mybir, bass, locals(), gelu_ok, dbgs)
            else:
                with tc.tile_pool(name="stub", bufs=1) as stub:
                    zz = stub.tile([1, BL], mybir.dt.float32)
                    nc.vector.memset(zz[:], 0.0)
                    nc.sync.dma_start(out=out32[:], in_=zz[:])
    nc.compile()
    return nc


def _seq_phase(nc, tc, mybir, bass, env, gelu_ok=True, dbgs=None):
    dbgs = dbgs or {}
    import concourse.tile as tile
    f32 = mybir.dt.float32
    bf16 = mybir.dt.bfloat16
    i32 = mybir.dt.int32
    i16 = mybir.dt.int16
    AF = mybir.ActivationFunctionType
    OP = mybir.AluOpType
    hfull = env["hfull"]
    ident = env["ident"]
    onesrow = env["onesrow"]
    onescol = env["onescol"]
    out32 = env["out32"]

    with (tc.tile_pool(name="scst", bufs=1) as scst,
          tc.tile_pool(name="outp", bufs=1) as outp):
        # ---- load seq constants
        def load(name, shape, dt):
            t = scst.tile(shape, dt, tag=name, name=name)
            nc.sync.dma_start(out=t[:], in_=env[name][:])
            return t

        gp1T = scst.tile([128, 2 * BL], f32, tag="gp1T", name="gp1T")
        nc.sync.dma_start(out=gp1T[:],
                          in_=env["gp1T"][:].rearrange("p c b -> p (c b)"))
        bfT = scst.tile([128, 2 * BL], f32, tag="bfT", name="bfT")
        nc.sync.dma_start(out=bfT[:],
                          in_=env["bfT"][:].rearrange("p c b -> p (c b)"))
        tridx = load("trajidx", [128, BL], i32)
        iddyn = load("iddyn", [49, BL * T], bf16)
        def loadl(lst, i, name, shape, dt):
            t = scst.tile(shape, dt, tag=name, name=name)
            nc.sync.dma_start(out=t[:], in_=lst[i][:])
            return t

        wifs = [loadl(env["wif"], i, f"wif{i}",
                      [128 if i < 2 else 49, 3 * H], bf16) for i in range(3)]
        wibs = [loadl(env["wib"], i, f"wib{i}",
                      [128 if i < 2 else 49, 3 * H], bf16) for i in range(3)]
        whfs = scst.tile([128, 2, 3 * H], bf16, tag="whfs")
        whbs = scst.tile([128, 2, 3 * H], bf16, tag="whbs")
        for k in range(2):
            nc.sync.dma_start(out=whfs[:, k, :],
                              in_=env["whf"][k * 128:(k + 1) * 128, :])
            nc.sync.dma_start(out=whbs[:, k, :],
                              in_=env["whb"][k * 128:(k + 1) * 128, :])
        aws = [loadl(env["aw"], i, f"aw{i}", [128 if i < 4 else 17, H],
                     bf16) for i in range(5)]
        aW2s = scst.tile([128, 2], bf16, tag="aW2s")
        nc.sync.dma_start(
            out=aW2s[:], in_=env["aW2"][:, 0].rearrange("(k p) -> p k", k=2))
        lnTs = load("lnT", [128, 4], f32)
        lnbTs = load("lnbT", [128, 4], f32)
        dynbs = load("dynb", [DYN + 1, BL], bf16)
        gws = [loadl(env["gw"], i, f"gw{i}", [128 if i < 8 else 17, 2 * H],
                     bf16) for i in range(9)]
        fws = [loadl(env["fw"], i, f"fw{i}", [128 if i < 4 else 17, H],
                     bf16) for i in range(5)]
        fW2s = scst.tile([128, 2], bf16, tag="fW2s")
        nc.sync.dma_start(
            out=fW2s[:], in_=env["fW2"][:, 0].rearrange("(k p) -> p k", k=2))

        apgxs = scst.tile([128, T * 6], i16, tag="apgxs")
        apgos = scst.tile([128, T * BL // 16], i16, tag="apgos")
        for g8 in range(8):
            nc.sync.dma_start(out=apgxs[g8 * 16:(g8 + 1) * 16, :],
                              in_=env["apgx"][:])
            nc.sync.dma_start(out=apgos[g8 * 16:(g8 + 1) * 16, :],
                              in_=env["apgo"][:])

        # maskrep [128, T*BL] f32 + bf16 copy (stream maskrow slices)
        maskrep = scst.tile([128, T * BL], f32)
        maskrepb = scst.tile([128, T * BL], bf16)
        with (tc.tile_pool(name="mrp", bufs=2, space="PSUM") as mrp,
              tc.tile_pool(name="mrs", bufs=2) as mrs):
            for i in range(T * BL // 512):
                mro = mrs.tile([1, 512], f32, tag="mro")
                nc.sync.dma_start(out=mro[:],
                                  in_=env["maskrow"][:, i * 512:(i + 1) * 512])
                ps = mrp.tile([128, 512], f32, tag="mr")
                nc.tensor.matmul(out=ps[:], lhsT=onesrow[:], rhs=mro[:],
                                 start=True, stop=True)
                nc.vector.tensor_copy(out=maskrep[:, i * 512:(i + 1) * 512],
                                      in_=ps[:])
                nc.vector.tensor_copy(out=maskrepb[:, i * 512:(i + 1) * 512],
                                      in_=ps[:])

        def dump(nm, ap, shape):
            if nm not in dbgs:
                return
            with tc.tile_pool(name=f"dmp{nm}", bufs=1) as dp:
                tt_ = dp.tile(list(shape), mybir.dt.float32, tag=nm, name=nm)
                nc.vector.tensor_copy(out=tt_[:], in_=ap)
                nc.sync.dma_start(out=dbgs[nm][:], in_=tt_[:])
        env["dump"] = dump

        # persistent outputs / states
        outfT = outp.tile([128, T * 2 * BL], bf16)
        outrbT = outp.tile([128, T * 2 * BL], bf16)
        outbT = outp.tile([128, T * 2 * BL], bf16)
        hTs = {d: outp.tile([128, 2, BL], f32, tag=f"hT{d}", name=f"hT{d}")
               for d in "fb"}
        maskv = maskrep[:].rearrange("p (t b) -> p t b", b=BL)

        def xg_compute(wis, xg_sb, dump=None):
            with (
                tc.tile_pool(name="xgs", bufs=4) as xgs,
                tc.tile_pool(name="xgp", bufs=5, space="PSUM") as xgp,
                tc.tile_pool(name="trp2", bufs=2, space="PSUM") as trp2,
            ):
                xgv = xg_sb[:].rearrange("p (t b m) -> p m b t",
                                         b=BL, m=6)
                for b in range(BL):
                    xsb = xgs.tile([128, H], f32, tag="xsb")
                    nc.gpsimd.indirect_dma_start(
                        out=xsb[:], out_offset=None, in_=hfull[:],
                        in_offset=bass.IndirectOffsetOnAxis(
                            ap=tridx[:, b:b + 1], axis=0))
                    if dump is not None and b == 0:
                        dump("dxs", xsb[:], [128, H])
                    xfT = xgs.tile([128, 2, 128], bf16, tag="xfT")
                    for c in range(2):
                        tp = trp2.tile([128, 128], f32, tag="trx")
                        nc.tensor.transpose(
                            out=tp[:], in_=xsb[:, c * 128:(c + 1) * 128],
                            identity=ident[:])
                        nc.scalar.activation(
                            xfT[:, c, :], tp[:], AF.Identity,
                            scale=gp1T[:, c * BL + b:c * BL + b + 1],
                            bias=bfT[:, c * BL + b:c * BL + b + 1])
                    if dump is not None and b == 0:
                        dump("dxf", xfT[:].rearrange("p c t -> p (c t)"),
                             [128, 2 * 128])
                    for m in range(6):
                        ps = xgp.tile([128, 128], f32, tag="xg")
                        msl = slice(m * 128, (m + 1) * 128)
                        nc.tensor.matmul(out=ps[:], lhsT=wis[0][:, msl],
                                         rhs=xfT[:, 0, :], start=True,
                                         stop=False)
                        nc.tensor.matmul(out=ps[:], lhsT=wis[1][:, msl],
                                         rhs=xfT[:, 1, :], start=False,
                                         stop=False)
                        nc.tensor.matmul(
                            out=ps[:], lhsT=wis[2][:, msl],
                            rhs=iddyn[:, b * T:(b + 1) * T],
                            start=False, stop=True)
                        nc.vector.tensor_copy(out=xgv[:, m, b, :], in_=ps[:])

        def recurrence(d, whs, xg_sb, outT):
            hT = hTs[d]
            nc.vector.memset(hT[:], 0.0)
            outv = outT[:].rearrange("p (t b c) -> p t c b", b=BL, c=2)
            xgr = xg_sb[:].rearrange("p (n d2) -> p n d2", d2=2)
            xgv = xg_sb[:].rearrange("p (t b m) -> p t m b", b=BL, m=6)
            with (
                tc.tile_pool(name=f"rec{d}", bufs=3) as rp,
                tc.tile_pool(name=f"rpp{d}", bufs=2, space="PSUM") as rpp,
                tc.tile_pool(name=f"rgx{d}", bufs=3) as rgx,
            ):
                hbf = rp.tile([128, 2, BL], bf16, tag="hbf")
                nc.vector.memset(hbf[:], 0.0)
                for t in range(T):
                    gh = rpp.tile([128, 6, BL], f32, tag="gh")
                    for m in range(6):
                        msl = slice(m * 128, (m + 1) * 128)
                        for k in range(2):
                            nc.tensor.matmul(
                                out=gh[:, m, :], lhsT=whs[:, k, msl],
                                rhs=hbf[:, k, :], start=(k == 0),
                                stop=(k == 1))
                    if d == "f":
                        xsl_src = xgv[:, t]
                    else:
                        xgt = rgx.tile([128, 96, 2], bf16, tag="xgt")
                        nc.gpsimd.ap_gather(
                            out_ap=xgt[:], in_ap=xgr,
                            idxs_ap=apgxs[:, t * 6:(t + 1) * 6],
                            channels=128, num_elems=T * 96, d=2,
                            num_idxs=96)
                        xsl_src = xgt[:].rearrange(
                            "p (b m2) d2 -> p (m2 d2) b", m2=3)
                    xsl = rp.tile([128, 6, BL], f32, tag="xsl")
                    nc.vector.tensor_copy(out=xsl[:], in_=xsl_src)
                    a1 = rp.tile([128, 4, BL], f32, tag="a1")
                    nc.vector.tensor_tensor(out=a1[:], in0=xsl[:, 0:4, :],
                                            in1=gh[:, 0:4, :], op=OP.add)
                    r = rp.tile([128, 2, BL], f32, tag="r")
                    nc.scalar.activation(r[:], a1[:, 0:2, :], AF.Sigmoid)
                    z1 = rp.tile([128, 2, BL], f32, tag="z1")
                    nc.scalar.activation(z1[:], a1[:, 2:4, :], AF.Sigmoid,
                                         scale=-1.0)
                    rn = rp.tile([128, 2, BL], f32, tag="rn")
                    nc.vector.tensor_tensor(out=rn[:], in0=r[:],
                                            in1=gh[:, 4:6, :], op=OP.mult)
                    nin = rp.tile([128, 2, BL], f32, tag="nin")
                    nc.vector.tensor_tensor(out=nin[:], in0=xsl[:, 4:6, :],
                                            in1=rn[:], op=OP.add)
                    n_ = rp.tile([128, 2, BL], f32, tag="n_")
                    nc.scalar.activation(n_[:], nin[:], AF.Tanh)
                    e1 = rp.tile([128, 2, BL], f32, tag="e1")
                    nc.vector.tensor_tensor(out=e1[:], in0=n_[:],
                                            in1=hT[:], op=OP.subtract)
                    zm = rp.tile([128, 2, BL], f32, tag="zm")
                    nc.vector.tensor_tensor(
                        out=zm[:], in0=z1[:],
                        in1=maskv[:, t].unsqueeze(1)
                        .to_broadcast([128, 2, BL]), op=OP.mult)
                    e2 = rp.tile([128, 2, BL], f32, tag="e2")
                    nc.vector.tensor_tensor(out=e2[:], in0=e1[:],
                                            in1=zm[:], op=OP.mult)
                    nc.vector.tensor_tensor(out=hT[:], in0=hT[:],
                                            in1=e2[:], op=OP.add)
                    nc.vector.tensor_tensor(
                        out=outv[:, t], in0=hT[:],
                        in1=maskv[:, t].unsqueeze(1)
                        .to_broadcast([128, 2, BL]), op=OP.mult)
                    nc.vector.tensor_copy(out=hbf[:], in_=hT[:])

        with tc.tile_pool(name="xgpool", bufs=1) as xgpool:
            xg_sb = xgpool.tile([128, T * BL * 6], bf16, tag="xg")
            xg_compute(wifs, xg_sb, dump)
            dump("diddyn", iddyn[:, 0:128], [49, 128])
            dump("dxg", xg_sb[:].rearrange("p (t b m) -> p t m b", b=BL,
                                           m=6)[:, 0], [128, 6, BL])
            recurrence("f", whfs, xg_sb, outfT)
            dump("dof", outfT[:, 0:256], [128, 256])
            dump("dhtf", hTs["f"][:], [128, 2, BL])
        with tc.tile_pool(name="xgpool2", bufs=1) as xgpool2:
            xg_sb2 = xgpool2.tile([128, T * BL * 6], bf16, tag="xg2")
            xg_compute(wibs, xg_sb2)
            dump("dxgb", xg_sb2[:].rearrange("p (t b m) -> p t m b", b=BL,
                                             m=6)[:, 0], [128, 6, BL])
            recurrence("b", whbs, xg_sb2, outrbT)
            dump("dorb", outrbT[:, 0:256], [128, 256])
            dump("dhtb", hTs["b"][:], [128, 2, BL])

        # un-reverse out_rb -> outbT, then mask
        nc.gpsimd.ap_gather(
            out_ap=outbT[:].rearrange("p (n d2) -> p n d2", d2=2),
            in_ap=outrbT[:].rearrange("p (n d2) -> p n d2", d2=2),
            idxs_ap=apgos[:], channels=128, num_elems=T * BL, d=2,
            num_idxs=T * BL)
        nc.vector.tensor_tensor(
            out=outbT[:].rearrange("p (t b c) -> p t b c", b=BL, c=2),
            in0=outbT[:].rearrange("p (t b c) -> p t b c", b=BL, c=2),
            in1=maskrepb[:].rearrange("p (t b) -> p t b", b=BL)
            .unsqueeze(3).to_broadcast([128, T, BL, 2]),
            op=OP.mult)

        dump("dob", outbT[:, 0:256], [128, 256])
        _attn_head(nc, tc, mybir, env, locals(), gelu_ok)


def _attn_head(nc, tc, mybir, env, sv, gelu_ok=True):
    f32 = mybir.dt.float32
    bf16 = mybir.dt.bfloat16
    AF = mybir.ActivationFunctionType
    OP = mybir.AluOpType
    outfT, outbT = sv["outfT"], sv["outbT"]
    hTs = sv["hTs"]
    aws, aW2s = sv["aws"], sv["aW2s"]
    lnTs, lnbTs, dynbs = sv["lnTs"], sv["lnbTs"], sv["dynbs"]
    gws, fws, fW2s = sv["gws"], sv["fws"], sv["fW2s"]
    onesrow, onescol, ident = (env["onesrow"], env["onescol"], env["ident"])
    out32 = env["out32"]
    NB = T * BL  # 4096

    with (
        tc.tile_pool(name="att", bufs=2) as at,
        tc.tile_pool(name="attc", bufs=1) as atc,
        tc.tile_pool(name="atp", bufs=2, space="PSUM") as atp,
    ):
        # rhs views: col = t*64 + b*2 + c
        ofv = outfT[:].rearrange("p (t b c) -> p c t b", b=BL, c=2)
        obv = outbT[:].rearrange("p (t b c) -> p c t b", b=BL, c=2)
        dynv = dynbs[:].unsqueeze(1)
        s1T = atc.tile([128, 2, NB], bf16)   # col t*32+b
        for c2 in range(2):
            csl = slice(c2 * 128, (c2 + 1) * 128)
            for nt in range(8):
                ts = slice(nt * 16, (nt + 1) * 16)
                ps = atp.tile([128, 512], f32, tag="s1")
                nc.tensor.matmul(out=ps[:], lhsT=aws[0][:, csl],
                                 rhs=ofv[:, 0, ts, :], start=True, stop=False)
                nc.tensor.matmul(out=ps[:], lhsT=aws[1][:, csl],
                                 rhs=ofv[:, 1, ts, :], start=False, stop=False)
                nc.tensor.matmul(out=ps[:], lhsT=aws[2][:, csl],
                                 rhs=obv[:, 0, ts, :], start=False, stop=False)
                nc.tensor.matmul(out=ps[:], lhsT=aws[3][:, csl],
                                 rhs=obv[:, 1, ts, :], start=False, stop=False)
                nc.tensor.matmul(out=ps[:], lhsT=aws[4][:, csl],
                                 rhs=dynv.to_broadcast([DYN + 1, 16, BL]),
                                 start=False, stop=True)
                nc.scalar.activation(
                    s1T[:, c2, nt * 512:(nt + 1) * 512], ps[:], AF.Tanh)
        # scores [1, NB] + negrow (streamed, straight to DRAM scd)
        # softmax over t (rows b): bounce through DRAM to repartition
        with tc.tile_pool(name="atd", bufs=1, space="DRAM") as atd:
            scd = atd.tile([T * BL], f32, tag="scd")
            ald = atd.tile([T * BL], f32, tag="ald")
            _softmax_ctx(nc, tc, mybir, env, sv, locals(), gelu_ok)


def _softmax_ctx(nc, tc, mybir, env, sv, av, gelu_ok=True):
    f32 = mybir.dt.float32
    bf16 = mybir.dt.bfloat16
    AF = mybir.ActivationFunctionType
    OP = mybir.AluOpType
    at, atc, atp = av["at"], av["atc"], av["atp"]
    scd, ald = av["scd"], av["ald"]
    s1T, aW2s = av["s1T"], av["aW2s"]
    ofv, obv = av["ofv"], av["obv"]
    dynbs = av["dynbs"]
    hTs = sv["hTs"]
    gws, fws, fW2s = sv["gws"], sv["fws"], sv["fW2s"]
    lnTs, lnbTs = sv["lnTs"], sv["lnbTs"]
    ident, onesrow, onescol = env["ident"], env["onesrow"], env["onescol"]
    out32 = env["out32"]
    NB = T * BL

    # scores per 512-slice: matmul + negrow add -> scd DRAM
    for nt8 in range(8):
        nsl = slice(nt8 * 512, (nt8 + 1) * 512)
        ps = atp.tile([1, 512], f32, tag="sc")
        for k in range(2):
            nc.tensor.matmul(out=ps[:], lhsT=aW2s[:, k:k + 1],
                             rhs=s1T[:, k, nsl], start=(k == 0),
                             stop=(k == 1))
        ngs = at.tile([1, 512], f32, tag="ngs")
        nc.sync.dma_start(out=ngs[:], in_=env["negrow"][:, nsl])
        sc5 = at.tile([1, 512], f32, tag="sc5")
        nc.vector.tensor_tensor(out=sc5[:], in0=ps[:], in1=ngs[:], op=OP.add)
        nc.sync.dma_start(out=scd[nsl], in_=sc5[:])
    dump = env.get("dump") if isinstance(env, dict) else None
    if dump is None:
        dump = sv.get("dump", lambda *a: None)
    dump("ds1", s1T[:, 0, 0:512], [128, 512])
    scb = at.tile([BL, T], f32, tag="scb")
    nc.sync.dma_start(out=scb[:],
                      in_=scd[:].rearrange("(t b) -> b t", b=BL))
    dump("dscb", scb[:], [BL, T])
    mx = at.tile([BL, 1], f32, tag="mx")
    nc.vector.tensor_reduce(out=mx[:], in_=scb[:],
                            axis=mybir.AxisListType.X, op=OP.max)
    nmx = at.tile([BL, 1], f32, tag="nmx")
    nc.vector.tensor_scalar_mul(out=nmx[:], in0=mx[:], scalar1=-1.0)
    ex = at.tile([BL, T], f32, tag="ex")
    nc.scalar.activation(ex[:], scb[:], AF.Exp, bias=nmx[:])
    sm = at.tile([BL, 1], f32, tag="sm")
    nc.vector.tensor_reduce(out=sm[:], in_=ex[:],
                            axis=mybir.AxisListType.X, op=OP.add)
    rs = at.tile([BL, 1], f32, tag="rs")
    nc.vector.reciprocal(out=rs[:], in_=sm[:])
    alph = at.tile([BL, T], f32, tag="alph")
    nc.vector.tensor_tensor(out=alph[:], in0=ex[:],
                            in1=rs[:].to_broadcast([BL, T]), op=OP.mult)
    # alpha -> [1, NB] row (t*32+b) via transpose + dma flatten
    aps_ = atp.tile([128, BL], f32, tag="at")
    nc.tensor.transpose(out=aps_[:, 0:BL], in_=alph[:],
                        identity=ident[0:BL, 0:BL])
    alT = at.tile([128, BL], f32, tag="alT")
    nc.vector.tensor_copy(out=alT[:], in_=aps_[:])
    nc.sync.dma_start(out=ald[:].rearrange("(t b) -> t b", b=BL),
                      in_=alT[:])
    dump("dal", alph[:], [BL, T])
    # alrep = ones x alpha-row (bf16), streamed from ald
    alrep = atc.tile([128, NB], bf16)
    for i in range(NB // 512):
        al5 = at.tile([1, 512], f32, tag="al5")
        nc.sync.dma_start(out=al5[:],
                          in_=ald[i * 512:(i + 1) * 512].unsqueeze(0))
        ps = atp.tile([128, 512], f32, tag="s1")
        nc.tensor.matmul(out=ps[:], lhsT=onesrow[:], rhs=al5[:],
                         start=True, stop=True)
        nc.vector.tensor_copy(out=alrep[:, i * 512:(i + 1) * 512],
                              in_=ps[:])
    # context ctxT [128, 4, BL] f32
    alv = alrep[:].rearrange("p (t b) -> p b t", b=BL)
    ctxT = atc.tile([128, 4, BL], f32)
    ctmp = at.tile([128, BL, T], bf16, tag="ctmp")
    for j, (ov, c) in enumerate([(ofv, 0), (ofv, 1), (obv, 0), (obv, 1)]):
        src = ov[:, c].rearrange("p t b -> p b t")
        nc.vector.tensor_tensor(out=ctmp[:], in0=src, in1=alv,
                                op=OP.mult)
        nc.vector.tensor_reduce(out=ctxT[:, j, :], in_=ctmp[:],
                                axis=mybir.AxisListType.X, op=OP.add)
    dump("dctx", ctxT[:], [128, 4, BL])
    ctxb = atc.tile([128, 4, BL], bf16)
    nc.vector.tensor_copy(out=ctxb[:], in_=ctxT[:])
    # layernorm of h_last = [hT_f, hT_b]
    hcat = atc.tile([128, 4, BL], f32)
    nc.vector.tensor_copy(out=hcat[:, 0:2, :], in_=hTs["f"][:])
    nc.vector.tensor_copy(out=hcat[:, 2:4, :], in_=hTs["b"][:])
    sq = at.tile([128, 4, BL], f32, tag="sq")
    nc.scalar.square(sq[:], hcat[:])
    psm = atp.tile([1, 4, BL], f32, tag="ln")
    nc.tensor.matmul(out=psm[:], lhsT=onescol[:], rhs=hcat[:],
                     start=True, stop=True)
    mu = at.tile([1, BL], f32, tag="mu")
    nc.vector.tensor_reduce(
        out=mu[:], in_=psm[:].rearrange("one c b -> one b c"),
        axis=mybir.AxisListType.X, op=OP.add)
    nc.vector.tensor_scalar_mul(out=mu[:], in0=mu[:], scalar1=1.0 / 512)
    psm2 = atp.tile([1, 4, BL], f32, tag="ln")
    nc.tensor.matmul(out=psm2[:], lhsT=onescol[:], rhs=sq[:],
                     start=True, stop=True)
    m2 = at.tile([1, BL], f32, tag="m2")
    nc.vector.tensor_reduce(
        out=m2[:], in_=psm2[:].rearrange("one c b -> one b c"),
        axis=mybir.AxisListType.X, op=OP.add)
    nc.vector.tensor_scalar_mul(out=m2[:], in0=m2[:], scalar1=1.0 / 512)
    msq = at.tile([1, BL], f32, tag="msq")
    nc.vector.tensor_tensor(out=msq[:], in0=mu[:], in1=mu[:], op=OP.mult)
    var = at.tile([1, BL], f32, tag="var")
    nc.vector.tensor_tensor(out=var[:], in0=m2[:], in1=msq[:],
                            op=OP.subtract)
    nc.vector.tensor_scalar_add(out=var[:], in0=var[:], scalar1=1e-5)
    sd = at.tile([1, BL], f32, tag="sd")
    nc.scalar.sqrt(sd[:], var[:])
    rstd = at.tile([1, BL], f32, tag="rstd")
    nc.vector.reciprocal(out=rstd[:], in_=sd[:])
    # broadcast mu/rstd to [128, BL]
    murep = at.tile([128, BL], f32, tag="murep")
    rsrep = at.tile([128, BL], f32, tag="rsrep")
    for (row, rep) in ((mu, murep), (rstd, rsrep)):
        ps = atp.tile([128, BL], f32, tag="at")
        nc.tensor.matmul(out=ps[:], lhsT=onesrow[:], rhs=row[:],
                         start=True, stop=True)
        nc.vector.tensor_copy(out=rep[:], in_=ps[:])
    xc = at.tile([128, 4, BL], f32, tag="xc")
    nc.vector.tensor_tensor(
        out=xc[:], in0=hcat[:],
        in1=murep[:].unsqueeze(1).to_broadcast([128, 4, BL]),
        op=OP.subtract)
    xn = at.tile([128, 4, BL], f32, tag="xn")
    nc.vector.tensor_tensor(
        out=xn[:], in0=xc[:],
        in1=rsrep[:].unsqueeze(1).to_broadcast([128, 4, BL]),
        op=OP.mult)
    rnl = atc.tile([128, 4, BL], f32)
    for c in range(4):
        nc.scalar.activation(rnl[:, c, :], xn[:, c, :], AF.Identity,
                             scale=lnTs[:, c:c + 1],
                             bias=lnbTs[:, c:c + 1])


# revision 34
# speedup vs baseline: 1.1656x; 1.1656x over previous
"""Trainium2 Bass kernel for nn_DAGRN; fully on-device 8-core SPMD."""
import sys
sys.path.insert(0, "/opt/trn_rl_repo")
import numpy as np


N = 50000
E = 800000
F = 64
H = 256
ID = 32
DYN = 16
B = 256
T = 128
STEPS = 3
W = 8
P = 6250          # nodes per core
PP = 6272         # padded (49*128)
NTILE = 49
NFULL = W * PP    # 50176
HALF = NFULL // 2  # 25088
BL = B // W       # 32 sequences per core


# ---------------------------------------------------------------- host prep

def to_bf16(x):
    import ml_dtypes
    return np.asarray(x, np.float32).astype(np.float16)


def prep_edges(edge_src, edge_dst):
    """Bucket edges by dst core, group by (dst_tile, src_half), pad each
    group to x128 with a structure shared across cores.
    Returns (NTg [49,2] int, esrc16 [W,16,EPAD//16] i16,
             dstrel [W,128,EPAD//128] f32)."""
    src = np.asarray(edge_src, np.int64)
    dst = np.asarray(edge_dst, np.int64)
    core = dst // P
    ldst = (dst - core * P).astype(np.int32)
    srcpad = ((src // P) * PP + src % P).astype(np.int32)
    half = (srcpad >= HALF).astype(np.int32)
    src16 = (srcpad - half * HALF).astype(np.int16)
    tile = ldst >> 7
    rel = (ldst & 127).astype(np.float32)

    key = ((core * NTILE + tile) * 2 + half).astype(np.int64)
    order = np.argsort(key, kind="stable")
    ks = key[order]
    counts = np.bincount(key, minlength=W * NTILE * 2).reshape(W, NTILE, 2)
    NTg = (np.ceil(counts / 128).astype(np.int64)).max(axis=0)  # [49,2]
    NTg = np.maximum(NTg, 1)  # avoid empty groups (psum never written)
    EPAD = int(NTg.sum()) * 128

    goff = np.zeros(NTILE * 2 + 1, np.int64)
    goff[1:] = np.cumsum(NTg.reshape(-1)) * 128  # group offsets in padded list

    # rank within group for each sorted edge
    gstart = np.zeros(W * NTILE * 2, np.int64)
    uniq, first = np.unique(ks, return_index=True)
    gstart[uniq] = first
    rank = np.arange(E) - gstart[ks]
    gl = ks - (ks // (NTILE * 2)) * (NTILE * 2)  # local group id (tile*2+half)
    dest = goff[gl] + rank
    co = ks // (NTILE * 2)

    esrc16 = np.zeros((W, EPAD), np.int16)
    dstrel = np.full((W, EPAD), -1.0, np.float32)
    esrc16[co, dest] = src16[order]
    dstrel[co, dest] = rel[order]

    esrc_w = np.ascontiguousarray(
        esrc16.reshape(W, EPAD // 16, 16).transpose(0, 2, 1))
    dstrel_w = np.ascontiguousarray(
        dstrel.reshape(W, EPAD // 128, 128).transpose(0, 2, 1))
    return NTg, esrc_w, dstrel_w


def wrap16(idx):
    """int16 [n] -> [16, n//16] wrapped layout."""
    idx = np.asarray(idx, np.int16)
    return np.ascontiguousarray(idx.reshape(-1, 16).T)


def prep_host(inp):
    """Build per-core in_maps + compile structure from full inputs."""
    f32 = np.float32
    g = lambda k: np.asarray(inp[k], f32)
    traj = np.asarray(inp["traj"], np.int64)
    lengths = np.asarray(inp["lengths"], np.int64)
    NTg, esrc_w, dstrel_w = prep_edges(inp["edge_src"], inp["edge_dst"])

    x0 = g("x0")
    dyn = g("dyn_feat")
    id_emb = g("id_emb")

    # x0t_aug per core: [65, PP] (row 64 = ones)
    x0p = np.zeros((W, PP, F), f32)
    x0p[:, :P] = x0.reshape(W, P, F)
    x0t = np.concatenate(
        [x0p.transpose(0, 2, 1), np.ones((W, 1, PP), f32)], 1)

    # GGNN weights (bf16 on device)
    Wz, Wr, Wn = g("Wz"), g("Wr"), g("Wn")
    bz, br, bn = g("bz"), g("br"), g("bn")
    wx_aug = np.concatenate([
        np.concatenate([Wz[:F], bz[None]], 0),
        np.concatenate([Wr[:F], br[None]], 0),
        np.concatenate([Wn[:F], bn[None]], 0)], 1)      # [65, 768]
    whc = np.concatenate([Wz[F:], Wr[F:], Wn[F:]], 1)   # [256, 768]
    wproj_aug = np.concatenate([g("Wproj"), g("bproj")[None]], 0)  # [65,256]

    # FiLM
    film = np.tanh((dyn @ g("film_W1") + g("film_b1")) @ g("film_W2")
                   + g("film_b2"))                       # [B, 512]
    gp1 = 1.0 + film[:, :H]
    bf = film[:, H:]
    gp1T = np.ascontiguousarray(
        gp1.reshape(W, BL, 2, 128).transpose(0, 3, 2, 1))  # [W,128,2,BL]
    bfT = np.ascontiguousarray(
        bf.reshape(W, BL, 2, 128).transpose(0, 3, 2, 1))

    # traj / mask / reversal
    trajpad = ((traj // P) * PP + traj % P).astype(np.int32)   # [B, T]
    tt = np.arange(T)
    mask = (tt[None, :] < lengths[:, None])                    # [B, T]
    rev = np.maximum(lengths[:, None] - 1 - tt[None, :], 0).astype(np.int64)
    trajidx = np.ascontiguousarray(
        trajpad.reshape(W, BL, T).transpose(0, 2, 1))          # [W,128,BL]
    # host-side time reversal for the bwd GRU: x_rev[b,t] = rnn_in[b,rev[b,t]]
    trajpad_b = np.take_along_axis(trajpad, rev, axis=1)       # [B, T]
    trajidxb = np.ascontiguousarray(
        trajpad_b.reshape(W, BL, T).transpose(0, 2, 1))        # [W,128,BL]
    traj_b = np.take_along_axis(traj, rev, axis=1)             # [B, T]

    maskrow = np.ascontiguousarray(
        mask.astype(f32).reshape(W, BL, T).transpose(0, 2, 1)
        .reshape(W, 1, T * BL))                                # col t*BL+b
    attn_b2 = float(np.asarray(inp["attn_b2"]).reshape(-1)[0])
    valid = (traj != 0)
    negb = np.where(valid, attn_b2, -1e9).astype(f32)
    negrow = np.ascontiguousarray(
        negb.reshape(W, BL, T).transpose(0, 2, 1).reshape(W, 1, T * BL))

    revl = rev.reshape(W, BL, T)                               # [W, b, s]
    # outs un-reverse idx (pairs over c? outs col = t*64 + c*32 + b,
    # pair=(b even,odd) same problem -> use col = t*64 + b*2 + c (c inner):
    # src pair = rev[b,t]*32 + b  (c'=pair of c=0,1, d=2)
    apgo = (revl * 32 + np.arange(BL)[None, :, None]) \
        .transpose(0, 2, 1).reshape(W, T * BL).astype(np.int16)  # [W,(t,b)]
    apgo_w = np.ascontiguousarray(
        apgo.reshape(W, T * BL // 16, 16).transpose(0, 2, 1))    # [W,16,256]

    # iddyn [49, BL*T] col = b*T + t (fwd and time-reversed for bwd)
    def make_iddyn(tj):
        idt = id_emb[tj]                                   # [B, T, 32]
        dd = np.zeros((W, 49, BL * T), f32)
        dd[:, :ID] = idt.reshape(W, BL, T, ID).transpose(0, 3, 1, 2) \
            .reshape(W, ID, BL * T)
        dd[:, ID:ID + DYN] = np.repeat(
            dyn.reshape(W, BL, DYN).transpose(0, 2, 1), T, axis=2
        ).reshape(W, DYN, BL * T)
        dd[:, ID + DYN] = 1.0
        return dd

    iddyn = make_iddyn(traj)
    iddynb = make_iddyn(traj_b)

    # GRU weights (bf16), bh folded into bi; z-gate (cols 256:512) negated
    # so r|z share one plain-sigmoid activation (z1 = sigmoid(-(xz+hz))).
    ZNEG = np.ones((1, 3 * H), f32)
    ZNEG[0, H:2 * H] = -1.0

    def wi_chunks(Wi, bi, bh):
        Wi = np.asarray(Wi, f32) * ZNEG
        bt = ((np.asarray(bi, f32) + np.asarray(bh, f32))[None]) * ZNEG
        k2 = np.concatenate([Wi[256:304], bt], 0)               # [49, 768]
        return to_bf16(Wi[0:128]), to_bf16(Wi[128:256]), to_bf16(k2)

    wif = wi_chunks(inp["gru_Wi_f"], inp["gru_bi_f"], inp["gru_bh_f"])
    wib = wi_chunks(inp["gru_Wi_b"], inp["gru_bi_b"], inp["gru_bh_b"])
    whf = to_bf16(np.asarray(inp["gru_Wh_f"], f32) * ZNEG)   # [256, 768]
    whb = to_bf16(np.asarray(inp["gru_Wh_b"], f32) * ZNEG)

    # attention W1 chunks [out_f 2, out_b 2, dyn+bias 17]
    aW1 = g("attn_W1")
    a_k = [to_bf16(aW1[i * 128:(i + 1) * 128]) for i in range(4)] \
        + [to_bf16(np.concatenate([aW1[512:528], g("attn_b1")[None]], 0))]
    aW2 = to_bf16(g("attn_W2"))                       # [256, 1]

    lnT = np.ascontiguousarray(g("ln_g").reshape(4, 128).T)   # [128,4]
    lnbT = np.ascontiguousarray(g("ln_b").reshape(4, 128).T)
    dynb = np.concatenate(
        [dyn.reshape(W, BL, DYN).transpose(0, 2, 1),
         np.ones((W, 1, BL), f32)], 1)                # [W, 17, BL]

    gW = g("gate_W")                                  # [1040, 512]
    g_k = [to_bf16(gW[i * 128:(i + 1) * 128]) for i in range(8)] \
        + [to_bf16(np.concatenate([gW[1024:1040], g("gate_b")[None]], 0))]
    fW1 = g("fc_W1")                                  # [528, 256]
    f_k = [to_bf16(fW1[i * 128:(i + 1) * 128]) for i in range(4)] \
        + [to_bf16(np.concatenate([fW1[512:528], g("fc_b1")[None]], 0))]
    fW2 = to_bf16(g("fc_W2"))                         # [256, 1]
    fc_b2 = float(np.asarray(inp["fc_b2"]).reshape(-1)[0])

    in_maps = []
    for c in range(W):
        m = dict(
            x0t=to_bf16(x0t[c]), esrc=esrc_w[c],
            dstrel=to_bf16(dstrel_w[c]),
            wx_aug=to_bf16(wx_aug), whc=to_bf16(whc),
            wproj_aug=to_bf16(wproj_aug),
            gp1T=gp1T[c], bfT=bfT[c], trajidx=trajidx[c],
            trajidxb=trajidxb[c],
            maskrow=maskrow[c], negrow=negrow[c],
            apgo=apgo_w[c], iddyn=to_bf16(iddyn[c]),
            iddynb=to_bf16(iddynb[c]),
            wif0=wif[0], wif1=wif[1], wif2=wif[2],
            wib0=wib[0], wib1=wib[1], wib2=wib[2],
            whf=whf, whb=whb,
            aw0=a_k[0], aw1=a_k[1], aw2=a_k[2], aw3=a_k[3], aw4=a_k[4],
            aW2=aW2, lnT=lnT, lnbT=lnbT, dynb=to_bf16(dynb[c]),
            gw0=g_k[0], gw1=g_k[1], gw2=g_k[2], gw3=g_k[3], gw4=g_k[4],
            gw5=g_k[5], gw6=g_k[6], gw7=g_k[7], gw8=g_k[8],
            fw0=f_k[0], fw1=f_k[1], fw2=f_k[2], fw3=f_k[3], fw4=f_k[4],
            fW2=fW2,
        )
        in_maps.append(m)
    return NTg, in_maps, fc_b2


# ---------------------------------------------------------------- device program

def build_program(NTg, debug_h=False, seq=True, steps=STEPS, gelu_ok=True,
                  dbg=False, seq_parts="full", ggnn_parts="full"):
    import concourse.bacc as bacc
    import concourse.mybir as mybir
    import concourse.tile as tile
    import concourse.bass as bass
    from concourse.masks import make_identity

    f32 = mybir.dt.float32
    bf16 = mybir.dt.float16  # fp16: same PE/DVE speed as bf16, 4x finer mantissa
    i16 = mybir.dt.int16
    i32 = mybir.dt.int32
    AF = mybir.ActivationFunctionType
    OP = mybir.AluOpType
    NTg = np.asarray(NTg)
    ETILES = int(NTg.sum())
    EPAD = ETILES * 128
    MAXNT = int(NTg.max())

    nc = bacc.Bacc("TRN2", target_bir_lowering=False, debug=False,
                   num_devices=W)
    DT = lambda *a, **k: nc.dram_tensor(*a, **k)
    # --- inputs
    x0t = DT("x0t", [F + 1, PP], bf16, kind="ExternalInput")
    esrc = DT("esrc", [16, EPAD // 16], i16, kind="ExternalInput")
    dstrel = DT("dstrel", [128, ETILES], bf16, kind="ExternalInput")
    wx_aug = DT("wx_aug", [F + 1, 3 * H], bf16, kind="ExternalInput")
    whc = DT("whc", [H, 3 * H], bf16, kind="ExternalInput")
    wproj_aug = DT("wproj_aug", [F + 1, H], bf16, kind="ExternalInput")
    gp1T = DT("gp1T", [128, 2, BL], f32, kind="ExternalInput")
    bfT = DT("bfT", [128, 2, BL], f32, kind="ExternalInput")
    trajidx = DT("trajidx", [128, BL], i32, kind="ExternalInput")
    trajidxb = DT("trajidxb", [128, BL], i32, kind="ExternalInput")
    maskrow = DT("maskrow", [1, T * BL], f32, kind="ExternalInput")
    negrow = DT("negrow", [1, T * BL], f32, kind="ExternalInput")
    apgo = DT("apgo", [16, T * BL // 16], i16, kind="ExternalInput")
    iddyn = DT("iddyn", [F // 64 * 49, BL * T], bf16, kind="ExternalInput")
    iddynb = DT("iddynb", [F // 64 * 49, BL * T], bf16, kind="ExternalInput")
    wif = [DT(f"wif{i}", [128, 3 * H] if i < 2 else [49, 3 * H], bf16,
              kind="ExternalInput") for i in range(3)]
    wib = [DT(f"wib{i}", [128, 3 * H] if i < 2 else [49, 3 * H], bf16,
              kind="ExternalInput") for i in range(3)]
    whf = DT("whf", [H, 3 * H], bf16, kind="ExternalInput")
    whb = DT("whb", [H, 3 * H], bf16, kind="ExternalInput")
    aw = [DT(f"aw{i}", [128 if i < 4 else 17, H], bf16,
             kind="ExternalInput") for i in range(5)]
    aW2 = DT("aW2", [H, 1], bf16, kind="ExternalInput")
    lnT = DT("lnT", [128, 4], f32, kind="ExternalInput")
    lnbT = DT("lnbT", [128, 4], f32, kind="ExternalInput")
    dynb = DT("dynb", [DYN + 1, BL], bf16, kind="ExternalInput")
    gw = [DT(f"gw{i}", [128 if i < 8 else 17, 2 * H], bf16,
             kind="ExternalInput") for i in range(9)]
    fw = [DT(f"fw{i}", [128 if i < 4 else 17, H], bf16,
             kind="ExternalInput") for i in range(5)]
    fW2 = DT("fW2", [H, 1], bf16, kind="ExternalInput")
    # --- outputs
    out32 = DT("out32", [1, BL], f32, kind="ExternalOutput")
    dbgs = {}
    if dbg:
        for nm, shp in [("dxf", [128, 256]), ("dxg", [128, 192]),
                        ("dxgb", [128, 192]), ("dof", [128, 256]),
                        ("dorb", [128, 256]), ("dob", [128, 256]),
                        ("ds1", [128, 512]), ("dscb", [BL, T]),
                        ("dal", [BL, T]), ("dctx", [128, 128]),
                        ("drnl", [128, 128]), ("dzf", [128, 128]),
                        ("dhfc", [128, 64]), ("dhtf", [128, 64]),
                        ("dhtb", [128, 64]), ("dxs", [128, 256]),
                        ("diddyn", [49, 128])]:
            dbgs[nm] = DT(nm, shp, f32, kind="ExternalOutput")
    if debug_h:
        hdbg = DT("hdbg", [PP, H], bf16, kind="ExternalOutput")
    # --- internal dram
    hfull = DT("hfull", [NFULL, H], bf16, kind="Internal",
               addr_space="Shared")

    with tile.TileContext(nc) as tc:
        with (tc.tile_pool(name="cst", bufs=1) as cst,
              tc.tile_pool(name="dram", bufs=1, space="DRAM") as dramp):
            cstg_cm = tc.tile_pool(name="cstg", bufs=1)
            cstg = cstg_cm.__enter__()
            hshard = dramp.tile([PP, H], bf16)
            # constants
            ident = cst.tile([128, 128], f32)
            make_identity(nc, ident[:])
            identb = cst.tile([128, 128], bf16)
            nc.vector.tensor_copy(out=identb[:], in_=ident[:])
            iotai = cst.tile([128, 128], i32)
            nc.gpsimd.iota(iotai[:], pattern=[[1, 128]], base=0,
                           channel_multiplier=0)
            iotab = cst.tile([128, 128], bf16)
            nc.vector.tensor_copy(out=iotab[:], in_=iotai[:])
            onesrow = cst.tile([1, 128], f32)
            nc.vector.memset(onesrow[:], 1.0)
            onescol = cst.tile([128, 1], f32)
            nc.vector.memset(onescol[:], 1.0)

            x0t_sb = cstg.tile([F + 1, PP], bf16)
            nc.sync.dma_start(out=x0t_sb[:], in_=x0t[:])
            wx_sb = cstg.tile([F + 1, 3 * H], bf16)
            nc.sync.dma_start(out=wx_sb[:], in_=wx_aug[:])
            whc_sb = [cstg.tile([128, 3 * H], bf16, tag=f"whc{k}",
                                name=f"whc{k}") for k in range(2)]
            for k in range(2):
                nc.sync.dma_start(out=whc_sb[k][:],
                                  in_=whc[k * 128:(k + 1) * 128, :])
            wproj_sb = cstg.tile([F + 1, H], bf16)
            nc.sync.dma_start(out=wproj_sb[:], in_=wproj_aug[:])
            esrc_sb = cstg.tile([128, EPAD // 16], i16)
            for g8 in range(8):
                nc.sync.dma_start(out=esrc_sb[g8 * 16:(g8 + 1) * 16, :],
                                  in_=esrc[:])
            dstrel_sb = cstg.tile([128, ETILES], bf16)
            nc.sync.dma_start(out=dstrel_sb[:], in_=dstrel[:])

            # ---------------- projection: h0 = tanh(x0 @ Wproj + b)
            with (
                tc.tile_pool(name="pj", bufs=3, space="PSUM") as pj,
                tc.tile_pool(name="sprj", bufs=3) as sprj,
            ):
                for t in range(NTILE):
                    ps = pj.tile([128, H], f32, tag="pj")
                    nc.tensor.matmul(
                        out=ps[:], lhsT=x0t_sb[:, t * 128:(t + 1) * 128],
                        rhs=wproj_sb[:], start=True, stop=True)
                    h0 = sprj.tile([128, H], bf16, tag="h0")
                    nc.scalar.activation(h0[:], ps[:], AF.Tanh)
                    nc.sync.dma_start(out=hshard[t * 128:(t + 1) * 128, :],
                                      in_=h0[:])
            nc.gpsimd.collective_compute(
                "AllGather", mybir.AluOpType.bypass,
                replica_groups=[list(range(W))],
                ins=[hshard[:]], outs=[hfull[:]])

            # ---------------- GGNN steps
            for s in range(steps):
                with (
                    tc.tile_pool(name=f"agp{s}", bufs=2, space="PSUM") as agp,
                    tc.tile_pool(name=f"gtp{s}", bufs=3, space="PSUM") as gtp,
                    tc.tile_pool(name=f"trp{s}", bufs=2, space="PSUM") as trp,
                    tc.tile_pool(name=f"sg{s}", bufs=3) as sg,
                    tc.tile_pool(name=f"sd{s}", bufs=2) as sd,
                ):
                    off16 = 0
                    offt = 0
                    for t in range(NTILE):
                        agg_ps = agp.tile([128, H], f32, tag="agg")
                        ntl, nth = int(NTg[t][0]), int(NTg[t][1])
                        first = True
                        for hx, nt in ((0, ntl), (1, nth)):
                            if nt == 0:
                                continue
                            msgs = sg.tile([128, MAXNT, H], bf16, tag="msgs")
                            src_half = (hfull[0:HALF, :] if hx == 0
                                        else hfull[HALF:NFULL, :])
                            # dma_gather dies above 1024 idxs per instr
                            if "nogather" not in ggnn_parts:
                                for g0 in range(0, nt, 8):
                                    gnt = min(8, nt - g0)
                                    nc.gpsimd.dma_gather(
                                        out_ap=msgs[:, g0:g0 + gnt, :],
                                        in_ap=src_half,
                                        idxs_ap=esrc_sb[
                                            :, off16 + g0 * 8:
                                            off16 + (g0 + gnt) * 8],
                                        num_idxs=gnt * 128,
                                        num_idxs_reg=gnt * 128,
                                        elem_size=H)
                            sstr = sg.tile([128, MAXNT, 128], bf16,
                                           tag="sstr")
                            if "nosstr" not in ggnn_parts:
                                nc.vector.tensor_tensor(
                                    out=sstr[:, 0:nt, :],
                                    in0=dstrel_sb[:, offt:offt + nt]
                                    .unsqueeze(2).to_broadcast([128, nt, 128]),
                                    in1=iotab[:].unsqueeze(1)
                                    .to_broadcast([128, nt, 128]),
                                    op=OP.is_equal)
                            if "noscatter" not in ggnn_parts:
                                for i in range(nt):
                                    last = ((hx == 1 or nth == 0)
                                            and i == nt - 1)
                                    nc.tensor.matmul(
                                        out=agg_ps[:], lhsT=sstr[:, i, :],
                                        rhs=msgs[:, i, :],
                                        start=first, stop=last)
                                    first = False
                            off16 += nt * 8
                            offt += nt
                        # dense update for this node tile
                        agg = sd.tile([128, H], f32, tag="agg_sb")
                        if "noscatter" not in ggnn_parts:
                            nc.vector.tensor_copy(out=agg[:], in_=agg_ps[:])
                        else:
                            nc.vector.memset(agg[:], 0.0)
                        if "nodense" in ggnn_parts:
                            hnz = sd.tile([128, H], bf16, tag="hnz")
                            nc.vector.tensor_copy(out=hnz[:], in_=agg[:])
                            nc.sync.dma_start(
                                out=hshard[t * 128:(t + 1) * 128, :],
                                in_=hnz[:])
                            continue
                        aggT = sd.tile([128, 2, 128], bf16, tag="aggT")
                        for c in range(2):
                            tp = trp.tile([128, 128], f32, tag="tr")
                            nc.tensor.transpose(
                                out=tp[:], in_=agg[:, c * 128:(c + 1) * 128],
                                identity=ident[:])
                            nc.vector.tensor_copy(out=aggT[:, c, :], in_=tp[:])
                        xsl = x0t_sb[:, t * 128:(t + 1) * 128]
                        zps = gtp.tile([128, H], f32, tag="gate")
                        nc.tensor.matmul(out=zps[:], lhsT=xsl,
                                         rhs=wx_sb[:, 0:H], start=True,
                                         stop=False)
                        for c in range(2):
                            nc.tensor.matmul(
                                out=zps[:], lhsT=aggT[:, c, :],
                                rhs=whc_sb[c][:, 0:H], start=False,
                                stop=(c == 1))
                        zg = sd.tile([128, H], f32, tag="zg")
                        nc.scalar.activation(zg[:], zps[:], AF.Sigmoid)
                        rps = gtp.tile([128, H], f32, tag="gate")
                        nc.tensor.matmul(out=rps[:], lhsT=xsl,
                                         rhs=wx_sb[:, H:2 * H], start=True,
                                         stop=False)
                        for c in range(2):
                            nc.tensor.matmul(
                                out=rps[:], lhsT=aggT[:, c, :],
                                rhs=whc_sb[c][:, H:2 * H], start=False,
                                stop=(c == 1))
                        rg = sd.tile([128, H], f32, tag="rg")
                        nc.scalar.activation(rg[:], rps[:], AF.Sigmoid)
                        rh = sd.tile([128, H], f32, tag="rh")
                        nc.vector.tensor_tensor(out=rh[:], in0=rg[:],
                                                in1=agg[:], op=OP.mult)
                        rhT = sd.tile([128, 2, 128], bf16, tag="rhT")
                        for c in range(2):
                            tp = trp.tile([128, 128], f32, tag="tr")
                            nc.tensor.transpose(
                                out=tp[:], in_=rh[:, c * 128:(c + 1) * 128],
                                identity=ident[:])
                            nc.vector.tensor_copy(out=rhT[:, c, :], in_=tp[:])
                        nps = gtp.tile([128, H], f32, tag="gate")
                        nc.tensor.matmul(out=nps[:], lhsT=xsl,
                                         rhs=wx_sb[:, 2 * H:3 * H],
                                         start=True, stop=False)
                        for c in range(2):
                            nc.tensor.matmul(
                                out=nps[:], lhsT=rhT[:, c, :],
                                rhs=whc_sb[c][:, 2 * H:3 * H], start=False,
                                stop=(c == 1))
                        ht = sd.tile([128, H], f32, tag="ht")
                        nc.scalar.activation(ht[:], nps[:], AF.Tanh)
                        d1 = sd.tile([128, H], f32, tag="d1")
                        nc.vector.tensor_tensor(out=d1[:], in0=ht[:],
                                                in1=agg[:], op=OP.subtract)
                        d2 = sd.tile([128, H], f32, tag="d2")
                        nc.vector.tensor_tensor(out=d2[:], in0=zg[:],
                                                in1=d1[:], op=OP.mult)
                        hnew = sd.tile([128, H], bf16, tag="hnew")
                        nc.vector.tensor_tensor(out=hnew[:], in0=agg[:],
                                                in1=d2[:], op=OP.add)
                        nc.sync.dma_start(
                            out=hshard[t * 128:(t + 1) * 128, :],
                            in_=hnew[:])
                nc.gpsimd.collective_compute(
                    "AllGather", mybir.AluOpType.bypass,
                    replica_groups=[list(range(W))],
                    ins=[hshard[:]], outs=[hfull[:]])

            if debug_h:
                with tc.tile_pool(name="dbg", bufs=2) as dbp:
                    for t in range(NTILE):
                        tb = dbp.tile([128, H], bf16, tag="dbg")
                        nc.sync.dma_start(
                            out=tb[:], in_=hshard[t * 128:(t + 1) * 128, :])
                        nc.sync.dma_start(
                            out=hdbg[t * 128:(t + 1) * 128, :], in_=tb[:])
            cstg_cm.__exit__(None, None, None)
            if seq:
                _seq_phase(nc, tc, mybir, bass, locals(), gelu_ok, dbgs,
                           parts=seq_parts)
            else:
                with tc.tile_pool(name="stub", bufs=1) as stub:
                    zz = stub.tile([1, BL], mybir.dt.float32)
                    nc.vector.memset(zz[:], 0.0)
                    nc.sync.dma_start(out=out32[:], in_=zz[:])
    nc.compile()
    return nc


def _seq_phase(nc, tc, mybir, bass, env, gelu_ok=True, dbgs=None,
               parts="full"):
    dbgs = dbgs or {}
    import concourse.tile as tile
    f32 = mybir.dt.float32
    bf16 = mybir.dt.float16  # fp16: same PE/DVE speed as bf16, 4x finer mantissa
    i32 = mybir.dt.int32
    i16 = mybir.dt.int16
    AF = mybir.ActivationFunctionType
    OP = mybir.AluOpType
    hfull = env["hfull"]
    ident = env["ident"]
    identb = env["identb"]
    onesrow = env["onesrow"]
    onescol = env["onescol"]
    out32 = env["out32"]

    with (tc.tile_pool(name="scst", bufs=1) as scst,
          tc.tile_pool(name="outp", bufs=1) as outp):
        # ---- load seq constants
        def load(name, shape, dt):
            t = scst.tile(shape, dt, tag=name, name=name)
            nc.sync.dma_start(out=t[:], in_=env[name][:])
            return t

        gp1T = scst.tile([128, 2 * BL], f32, tag="gp1T", name="gp1T")
        nc.sync.dma_start(out=gp1T[:],
                          in_=env["gp1T"][:].rearrange("p c b -> p (c b)"))
        bfT = scst.tile([128, 2 * BL], f32, tag="bfT", name="bfT")
        nc.sync.dma_start(out=bfT[:],
                          in_=env["bfT"][:].rearrange("p c b -> p (c b)"))
        tridx = load("trajidx", [128, BL], i32)
        tridxb = load("trajidxb", [128, BL], i32)

        def loadl(lst, i, name, shape, dt, pool=None):
            t = (pool or scst).tile(shape, dt, tag=name, name=name)
            nc.sync.dma_start(out=t[:], in_=lst[i][:])
            return t

        whfs = scst.tile([128, 2, 3 * H], bf16, tag="whfs")
        whbs = scst.tile([128, 2, 3 * H], bf16, tag="whbs")
        for k in range(2):
            nc.sync.dma_start(out=whfs[:, k, :],
                              in_=env["whf"][k * 128:(k + 1) * 128, :])
            nc.sync.dma_start(out=whbs[:, k, :],
                              in_=env["whb"][k * 128:(k + 1) * 128, :])

        apgos = scst.tile([128, T * BL // 16], i16, tag="apgos")
        for g8 in range(8):
            nc.sync.dma_start(out=apgos[g8 * 16:(g8 + 1) * 16, :],
                              in_=env["apgo"][:])

        # maskrep [128, T*BL] bf16 (stream maskrow slices)
        maskrepb = scst.tile([128, T * BL], bf16)
        with (tc.tile_pool(name="mrp", bufs=2, space="PSUM") as mrp,
              tc.tile_pool(name="mrs", bufs=2) as mrs):
            for i in range(T * BL // 512):
                mro = mrs.tile([1, 512], f32, tag="mro")
                nc.sync.dma_start(out=mro[:],
                                  in_=env["maskrow"][:, i * 512:(i + 1) * 512])
                ps = mrp.tile([128, 512], f32, tag="mr")
                nc.tensor.matmul(out=ps[:], lhsT=onesrow[:], rhs=mro[:],
                                 start=True, stop=True)
                nc.vector.tensor_copy(out=maskrepb[:, i * 512:(i + 1) * 512],
                                      in_=ps[:])

        def dump(nm, ap, shape):
            if nm not in dbgs:
                return
            with tc.tile_pool(name=f"dmp{nm}", bufs=1) as dp:
                tt_ = dp.tile(list(shape), mybir.dt.float32, tag=nm, name=nm)
                nc.vector.tensor_copy(out=tt_[:], in_=ap)
                nc.sync.dma_start(out=dbgs[nm][:], in_=tt_[:])
        env["dump"] = dump

        # persistent outputs / states
        outfT = outp.tile([128, T * 2 * BL], bf16)
        outrbT = outp.tile([128, T * 2 * BL], bf16)
        outbT = outp.tile([128, T * 2 * BL], bf16)
        hTs = {d: outp.tile([128, 2, BL], f32, tag=f"hT{d}", name=f"hT{d}")
               for d in "fb"}
        maskbv = maskrepb[:].rearrange("p (t b) -> p t b", b=BL)

        def xg_compute(wis, iddyn_t, tridx_t, xg_sb, dump=None):
            with (
                tc.tile_pool(name="xgs", bufs=4) as xgs,
                tc.tile_pool(name="xgp", bufs=5, space="PSUM") as xgp,
                tc.tile_pool(name="trp2", bufs=2, space="PSUM") as trp2,
            ):
                # xg col layout: t*(6*BL) + m*BL + b
                xgv = xg_sb[:].rearrange("p (t m b) -> p m b t",
                                         b=BL, m=6)
                for b in range(BL):
                    xsb = xgs.tile([128, H], bf16, tag="xsb")
                    nc.gpsimd.indirect_dma_start(
                        out=xsb[:], out_offset=None, in_=hfull[:],
                        in_offset=bass.IndirectOffsetOnAxis(
                            ap=tridx_t[:, b:b + 1], axis=0))
                    if dump is not None and b == 0:
                        dump("dxs", xsb[:], [128, H])
                    xfT = xgs.tile([128, 2, 128], bf16, tag="xfT")
                    for c in range(2):
                        tp = trp2.tile([128, 128], bf16, tag="trx")
                        nc.tensor.transpose(
                            out=tp[:], in_=xsb[:, c * 128:(c + 1) * 128],
                            identity=identb[:])
                        nc.scalar.activation(
                            xfT[:, c, :], tp[:], AF.Identity,
                            scale=gp1T[:, c * BL + b:c * BL + b + 1],
                            bias=bfT[:, c * BL + b:c * BL + b + 1])
                    if dump is not None and b == 0:
                        dump("dxf", xfT[:].rearrange("p c t -> p (c t)"),
                             [128, 2 * 128])
                    for m in range(6):
                        ps = xgp.tile([128, 128], f32, tag="xg")
                        msl = slice(m * 128, (m + 1) * 128)
                        nc.tensor.matmul(out=ps[:], lhsT=wis[0][:, msl],
                                         rhs=xfT[:, 0, :], start=True,
                                         stop=False)
                        nc.tensor.matmul(out=ps[:], lhsT=wis[1][:, msl],
                                         rhs=xfT[:, 1, :], start=False,
                                         stop=False)
                        nc.tensor.matmul(
                            out=ps[:], lhsT=wis[2][:, msl],
                            rhs=iddyn_t[:, b * T:(b + 1) * T],
                            start=False, stop=True)
                        nc.vector.tensor_copy(out=xgv[:, m, b, :], in_=ps[:])

        def recurrence2(xg_f, xg_b):
            """Interleaved fwd+bwd GRU; both xg buffers already in step
            order (bwd time-reversed on host). z-gate pre-negated so r|z
            share one sigmoid."""
            whss = {"f": whfs, "b": whbs}
            xgvs = {"f": xg_f[:].rearrange("p (t m b) -> p t m b",
                                           b=BL, m=6),
                    "b": xg_b[:].rearrange("p (t m b) -> p t m b",
                                           b=BL, m=6)}
            outvs = {"f": outfT[:].rearrange("p (t b c) -> p t c b",
                                             b=BL, c=2),
                     "b": outrbT[:].rearrange("p (t b c) -> p t c b",
                                              b=BL, c=2)}
            with (
                tc.tile_pool(name="recs", bufs=3) as rp,
                tc.tile_pool(name="rppf", bufs=2, space="PSUM") as rppf,
                tc.tile_pool(name="rppb", bufs=2, space="PSUM") as rppb,
                tc.tile_pool(name="rech", bufs=1) as rh_,
            ):
                rpps = {"f": rppf, "b": rppb}
                hbfs = {}
                for d in "fb":
                    nc.vector.memset(hTs[d][:], 0.0)
                    hbfs[d] = rh_.tile([128, 2, BL], bf16, tag=f"hbf{d}",
                                       name=f"hbf{d}")
                    nc.vector.memset(hbfs[d][:], 0.0)
                for t in range(T):
                    for d in "fb":
                        whs, hT, hbf = whss[d], hTs[d], hbfs[d]
                        xgv = xgvs[d]
                        gh = rpps[d].tile([128, 6, BL], f32, tag=f"gh{d}")
                        for m in range(6):
                            msl = slice(m * 128, (m + 1) * 128)
                            for k in range(2):
                                nc.tensor.matmul(
                                    out=gh[:, m, :], lhsT=whs[:, k, msl],
                                    rhs=hbf[:, k, :], start=(k == 0),
                                    stop=(k == 1))
                        a1 = rp.tile([128, 4, BL], f32, tag=f"a1{d}")
                        nc.vector.tensor_tensor(out=a1[:],
                                                in0=xgv[:, t, 0:4, :],
                                                in1=gh[:, 0:4, :], op=OP.add)
                        rz = rp.tile([128, 4, BL], f32, tag=f"rz{d}")
                        nc.scalar.activation(rz[:], a1[:], AF.Sigmoid)
                        rn = rp.tile([128, 2, BL], f32, tag=f"rn{d}")
                        nc.vector.tensor_tensor(out=rn[:], in0=rz[:, 0:2, :],
                                                in1=gh[:, 4:6, :],
                                                op=OP.mult)
                        nin = rp.tile([128, 2, BL], f32, tag=f"nin{d}")
                        nc.vector.tensor_tensor(out=nin[:],
                                                in0=xgv[:, t, 4:6, :],
                                                in1=rn[:], op=OP.add)
                        n_ = rp.tile([128, 2, BL], f32, tag=f"n_{d}")
                        nc.scalar.activation(n_[:], nin[:], AF.Tanh)
                        zm = rp.tile([128, 2, BL], f32, tag=f"zm{d}")
                        nc.gpsimd.tensor_tensor(
                            out=zm[:], in0=rz[:, 2:4, :],
                            in1=maskbv[:, t].unsqueeze(1)
                            .to_broadcast([128, 2, BL]), op=OP.mult)
                        e1 = rp.tile([128, 2, BL], f32, tag=f"e1{d}")
                        nc.vector.tensor_tensor(out=e1[:], in0=n_[:],
                                                in1=hT[:], op=OP.subtract)
                        e2 = rp.tile([128, 2, BL], f32, tag=f"e2{d}")
                        nc.vector.tensor_tensor(out=e2[:], in0=e1[:],
                                                in1=zm[:], op=OP.mult)
                        # hbf (bf16, feeds next-step matmul) on Pool in
                        # parallel with the f32 in-place update on DVE
                        nc.gpsimd.tensor_tensor(out=hbf[:], in0=hT[:],
                                                in1=e2[:], op=OP.add)
                        nc.vector.tensor_tensor(out=hT[:], in0=hT[:],
                                                in1=e2[:], op=OP.add)
                        nc.gpsimd.tensor_tensor(
                            out=outvs[d][:, t], in0=hT[:],
                            in1=maskbv[:, t].unsqueeze(1)
                            .to_broadcast([128, 2, BL]), op=OP.mult)

        with tc.tile_pool(name="xgpool", bufs=1) as xgpool:
            xg_sb = xgpool.tile([128, T * BL * 6], bf16, tag="xg")
            xg_sb2 = xgpool.tile([128, T * BL * 6], bf16, tag="xg2")
            # xg-only constants, released before the recurrence
            xgc_cm = tc.tile_pool(name="xgc", bufs=1)
            xgc = xgc_cm.__enter__()
            iddyn = xgc.tile([49, BL * T], bf16, tag="iddyn", name="iddyn")
            nc.sync.dma_start(out=iddyn[:], in_=env["iddyn"][:])
            iddynb = xgc.tile([49, BL * T], bf16, tag="iddynb",
                              name="iddynb")
            nc.sync.dma_start(out=iddynb[:], in_=env["iddynb"][:])
            wifs = [loadl(env["wif"], i, f"wif{i}",
                          [128 if i < 2 else 49, 3 * H], bf16, xgc)
                    for i in range(3)]
            wibs = [loadl(env["wib"], i, f"wib{i}",
                          [128 if i < 2 else 49, 3 * H], bf16, xgc)
                    for i in range(3)]
            xg_compute(wifs, iddyn, tridx, xg_sb, dump)
            xg_compute(wibs, iddynb, tridxb, xg_sb2)
            dump("diddyn", iddyn[:, 0:128], [49, 128])
            dump("dxg", xg_sb[:].rearrange("p (t m b) -> p t m b", b=BL,
                                           m=6)[:, 0], [128, 6, BL])
            xgc_cm.__exit__(None, None, None)
            if parts in ("rec", "full"):
                recurrence2(xg_sb, xg_sb2)
            dump("dof", outfT[:, 0:256], [128, 256])
            dump("dhtf", hTs["f"][:], [128, 2, BL])
            dump("dorb", outrbT[:, 0:256], [128, 256])
            dump("dhtb", hTs["b"][:], [128, 2, BL])

        # attention / head weights (loaded after the recurrence frees SBUF)
        aws = [loadl(env["aw"], i, f"aw{i}", [128 if i < 4 else 17, H],
                     bf16) for i in range(5)]
        aW2s = scst.tile([128, 2], bf16, tag="aW2s")
        nc.sync.dma_start(
            out=aW2s[:], in_=env["aW2"][:, 0].rearrange("(k p) -> p k", k=2))
        lnTs = load("lnT", [128, 4], f32)
        lnbTs = load("lnbT", [128, 4], f32)
        dynbs = load("dynb", [DYN + 1, BL], bf16)
        gws = [loadl(env["gw"], i, f"gw{i}", [128 if i < 8 else 17, 2 * H],
                     bf16) for i in range(9)]
        fws = [loadl(env["fw"], i, f"fw{i}", [128 if i < 4 else 17, H],
                     bf16) for i in range(5)]
        fW2s = scst.tile([128, 2], bf16, tag="fW2s")
        nc.sync.dma_start(
            out=fW2s[:], in_=env["fW2"][:, 0].rearrange("(k p) -> p k", k=2))

        if parts != "full":
            with tc.tile_pool(name="stub2", bufs=1) as stub2:
                zz = stub2.tile([1, BL], f32)
                nc.vector.memset(zz[:], 0.0)
                nc.sync.dma_start(out=env["out32"][:], in_=zz[:])
            return

        # un-reverse out_rb -> outbT, then mask
        nc.gpsimd.ap_gather(
            out_ap=outbT[:].rearrange("p (n d2) -> p n d2", d2=2),
            in_ap=outrbT[:].rearrange("p (n d2) -> p n d2", d2=2),
            idxs_ap=apgos[:], channels=128, num_elems=T * BL, d=2,
            num_idxs=T * BL)
        nc.vector.tensor_tensor(
            out=outbT[:].rearrange("p (t b c) -> p t b c", b=BL, c=2),
            in0=outbT[:].rearrange("p (t b c) -> p t b c", b=BL, c=2),
            in1=maskrepb[:].rearrange("p (t b) -> p t b", b=BL)
            .unsqueeze(3).to_broadcast([128, T, BL, 2]),
            op=OP.mult)

        dump("dob", outbT[:, 0:256], [128, 256])
        _attn_head(nc, tc, mybir, env, locals(), gelu_ok)


def _attn_head(nc, tc, mybir, env, sv, gelu_ok=True):
    f32 = mybir.dt.float32
    bf16 = mybir.dt.float16  # fp16: same PE/DVE speed as bf16, 4x finer mantissa
    AF = mybir.ActivationFunctionType
    OP = mybir.AluOpType
    outfT, outbT = sv["outfT"], sv["outbT"]
    hTs = sv["hTs"]
    aws, aW2s = sv["aws"], sv["aW2s"]
    lnTs, lnbTs, dynbs = sv["lnTs"], sv["lnbTs"], sv["dynbs"]
    gws, fws, fW2s = sv["gws"], sv["fws"], sv["fW2s"]
    onesrow, onescol, ident = (env["onesrow"], env["onescol"], env["ident"])
    out32 = env["out32"]
    NB = T * BL  # 4096

    with (
        tc.tile_pool(name="att", bufs=2) as at,
        tc.tile_pool(name="attc", bufs=1) as atc,
        tc.tile_pool(name="atp", bufs=2, space="PSUM") as atp,
    ):
        # rhs views: col = t*64 + b*2 + c
        ofv = outfT[:].rearrange("p (t b c) -> p c t b", b=BL, c=2)
        obv = outbT[:].rearrange("p (t b c) -> p c t b", b=BL, c=2)
        dynv = dynbs[:].unsqueeze(1)
        s1T = atc.tile([128, 2, NB], bf16)   # col t*32+b
        for c2 in range(2):
            csl = slice(c2 * 128, (c2 + 1) * 128)
            for nt in range(8):
                ts = slice(nt * 16, (nt + 1) * 16)
                ps = atp.tile([128, 512], f32, tag="s1")
                nc.tensor.matmul(out=ps[:], lhsT=aws[0][:, csl],
                                 rhs=ofv[:, 0, ts, :], start=True, stop=False)
                nc.tensor.matmul(out=ps[:], lhsT=aws[1][:, csl],
                                 rhs=ofv[:, 1, ts, :], start=False, stop=False)
                nc.tensor.matmul(out=ps[:], lhsT=aws[2][:, csl],
                                 rhs=obv[:, 0, ts, :], start=False, stop=False)
                nc.tensor.matmul(out=ps[:], lhsT=aws[3][:, csl],
                                 rhs=obv[:, 1, ts, :], start=False, stop=False)
                nc.tensor.matmul(out=ps[:], lhsT=aws[4][:, csl],
                                 rhs=dynv.to_broadcast([DYN + 1, 16, BL]),
                                 start=False, stop=True)
                nc.scalar.activation(
                    s1T[:, c2, nt * 512:(nt + 1) * 512], ps[:], AF.Tanh)
        # scores [1, NB] + negrow (streamed, straight to DRAM scd)
        # softmax over t (rows b): bounce through DRAM to repartition
        with tc.tile_pool(name="atd", bufs=1, space="DRAM") as atd:
            scd = atd.tile([T * BL], f32, tag="scd")
            ald = atd.tile([T * BL], f32, tag="ald")
            _softmax_ctx(nc, tc, mybir, env, sv, locals(), gelu_ok)


def _softmax_ctx(nc, tc, mybir, env, sv, av, gelu_ok=True):
    f32 = mybir.dt.float32
    bf16 = mybir.dt.float16  # fp16: same PE/DVE speed as bf16, 4x finer mantissa
    AF = mybir.ActivationFunctionType
    OP = mybir.AluOpType
    at, atc, atp = av["at"], av["atc"], av["atp"]
    scd, ald = av["scd"], av["ald"]
    s1T, aW2s = av["s1T"], av["aW2s"]
    ofv, obv = av["ofv"], av["obv"]
    dynbs = av["dynbs"]
    hTs = sv["hTs"]
    gws, fws, fW2s = sv["gws"], sv["fws"], sv["fW2s"]
    lnTs, lnbTs = sv["lnTs"], sv["lnbTs"]
    ident, onesrow, onescol = env["ident"], env["onesrow"], env["onescol"]
    out32 = env["out32"]
    NB = T * BL

    # scores per 512-slice: matmul + negrow add -> scd DRAM
    for nt8 in range(8):
        nsl = slice(nt8 * 512, (nt8 + 1) * 512)
        ps = atp.tile([1, 512], f32, tag="sc")
        for k in range(2):
            nc.tensor.matmul(out=ps[:], lhsT=aW2s[:, k:k + 1],
                             rhs=s1T[:, k, nsl], start=(k == 0),
                             stop=(k == 1))
        ngs = at.tile([1, 512], f32, tag="ngs")
        nc.sync.dma_start(out=ngs[:], in_=env["negrow"][:, nsl])
        sc5 = at.tile([1, 512], f32, tag="sc5")
        nc.vector.tensor_tensor(out=sc5[:], in0=ps[:], in1=ngs[:], op=OP.add)
        nc.sync.dma_start(out=scd[nsl], in_=sc5[:])
    dump = env.get("dump") if isinstance(env, dict) else None
    if dump is None:
        dump = sv.get("dump", lambda *a: None)
    dump("ds1", s1T[:, 0, 0:512], [128, 512])
    scb = at.tile([BL, T], f32, tag="scb")
    nc.sync.dma_start(out=scb[:],
                      in_=scd[:].rearrange("(t b) -> b t", b=BL))
    dump("dscb", scb[:], [BL, T])
    mx = at.tile([BL, 1], f32, tag="mx")
    nc.vector.tensor_reduce(out=mx[:], in_=scb[:],
                            axis=mybir.AxisListType.X, op=OP.max)
    nmx = at.tile([BL, 1], f32, tag="nmx")
    nc.vector.tensor_scalar_mul(out=nmx[:], in0=mx[:], scalar1=-1.0)
    ex = at.tile([BL, T], f32, tag="ex")
    nc.scalar.activation(ex[:], scb[:], AF.Exp, bias=nmx[:])
    sm = at.tile([BL, 1], f32, tag="sm")
    nc.vector.tensor_reduce(out=sm[:], in_=ex[:],
                            axis=mybir.AxisListType.X, op=OP.add)
    rs = at.tile([BL, 1], f32, tag="rs")
    nc.vector.reciprocal(out=rs[:], in_=sm[:])
    alph = at.tile([BL, T], f32, tag="alph")
    nc.vector.tensor_tensor(out=alph[:], in0=ex[:],
                            in1=rs[:].to_broadcast([BL, T]), op=OP.mult)
    # alpha -> [1, NB] row (t*32+b) via transpose + dma flatten
    aps_ = atp.tile([128, BL], f32, tag="at")
    nc.tensor.transpose(out=aps_[:, 0:BL], in_=alph[:],
                        identity=ident[0:BL, 0:BL])
    alT = at.tile([128, BL], f32, tag="alT")
    nc.vector.tensor_copy(out=alT[:], in_=aps_[:])
    nc.sync.dma_start(out=ald[:].rearrange("(t b) -> t b", b=BL),
                      in_=alT[:])
    dump("dal", alph[:], [BL, T])
    # alrep = ones x alpha-row (bf16), streamed from ald
    alrep = atc.tile([128, NB], bf16)
    for i in range(NB // 512):
        al5 = at.tile([1, 512], f32, tag="al5")
        nc.sync.dma_start(out=al5[:],
                          in_=ald[i * 512:(i + 1) * 512].unsqueeze(0))
        ps = atp.tile([128, 512], f32, tag="s1")
        nc.tensor.matmul(out=ps[:], lhsT=onesrow[:], rhs=al5[:],
                         start=True, stop=True)
        nc.vector.tensor_copy(out=alrep[:, i * 512:(i + 1) * 512],
                              in_=ps[:])
    # context ctxT [128, 4, BL] f32
    alv = alrep[:].rearrange("p (t b) -> p b t", b=BL)
    ctxT = atc.tile([128, 4, BL], f32)
    ctmp = at.tile([128, BL, T], bf16, tag="ctmp")
    for j, (ov, c) in enumerate([(ofv, 0), (ofv, 1), (obv, 0), (obv, 1)]):
        src = ov[:, c].rearrange("p t b -> p b t")
        nc.vector.tensor_tensor(out=ctmp[:], in0=src, in1=alv,
                                op=OP.mult)
        nc.vector.tensor_reduce(out=ctxT[:, j, :], in_=ctmp[:],
                                axis=mybir.AxisListType.X, op=OP.add)
    dump("dctx", ctxT[:], [128, 4, BL])
    ctxb = atc.tile([128, 4, BL], bf16)
    nc.vector.tensor_copy(out=ctxb[:], in_=ctxT[:])
    # layernorm of h_last = [hT_f, hT_b]
    hcat = atc.tile([128, 4, BL], f32)
    nc.vector.tensor_copy(out=hcat[:, 0:2, :], in_=hTs["f"][:])
    nc.vector.tensor_copy(out=hcat[:, 2:4, :], in_=hTs["b"][:])
    sq = at.tile([128, 4, BL], f32, tag="sq")
    nc.scalar.square(sq[:], hcat[:])
    psm = atp.tile([1, 4, BL], f32, tag="ln")
    nc.tensor.matmul(out=psm[:], lhsT=onescol[:], rhs=hcat[:],
                     start=True, stop=True)
    mu = at.tile([1, BL], f32, tag="mu")
    nc.vector.tensor_reduce(
        out=mu[:], in_=psm[:].rearrange("one c b -> one b c"),
        axis=mybir.AxisListType.X, op=OP.add)
    nc.vector.tensor_scalar_mul(out=mu[:], in0=mu[:], scalar1=1.0 / 512)
    psm2 = atp.tile([1, 4, BL], f32, tag="ln")
    nc.tensor.matmul(out=psm2[:], lhsT=onescol[:], rhs=sq[:],
                     start=True, stop=True)
    m2 = at.tile([1, BL], f32, tag="m2")
    nc.vector.tensor_reduce(
        out=m2[:], in_=psm2[:].rearrange("one c b -> one b c"),
        axis=mybir.AxisListType.X, op=OP.add)
    nc.vector.tensor_scalar_mul(out=m2[:], in0=m2[:], scalar1=1.0 / 512)
    msq = at.tile([1, BL], f32, tag="msq")
    nc.vector.tensor_tensor(out=msq[:], in0=mu[:], in1=mu[:], op=OP.mult)
    var = at.tile([1, BL], f32, tag="var")
    nc.vector.tensor_tensor(out=var[:], in0=m2[:], in1=msq[:],
                            op=OP.subtract)
    nc.vector.tensor_scalar_add(out=var[:], in0=var[:], scalar1=1e-5)
    sd = at.tile([1, BL], f32, tag="sd")
    nc.scalar.sqrt(sd[:], var[:])
    rstd = at.tile([1, BL], f32, tag="rstd")
    nc.vector.reciprocal(out=rstd[:], in_=sd[:])
    # broadcast mu/rstd to [128, BL]
    murep = at.tile([128, BL], f32, tag="murep")
    rsrep = at.tile([128, BL], f32, tag="rsrep")
    for (row, rep) in ((mu, murep), (rstd, rsrep)):
        ps = atp.tile([128, BL], f32, tag="at")
        nc.tensor.matmul(out=ps[:], lhsT=onesrow[:], rhs=row[:],
                         start=True, stop=True)
        nc.vector.tensor_copy(out=rep[:], in_=ps[:])
    xc = at.tile([128, 4, BL], f32, tag="xc")
    nc.vector.tensor_tensor(
        out=xc[:], in0=hcat[:],
        in1=murep[:].unsqueeze(1).to_broadcast([128, 4, BL]),
        op=OP.subtract)
    xn = at.tile([128, 4, BL], f32, tag="xn")
    nc.vector.tensor_tensor(
        out=xn[:], in0=xc[:],
        in1=rsrep[:].unsqueeze(1).to_broadcast([128, 4, BL]),
        op=OP.mult)
    rnl = atc.tile([128, 4, BL], f32)
    for c in range(4):
        nc.scalar.activation(rnl[:, c, :], xn[:, c, :], AF.Identity,
                             scale=lnTs[:, c:c + 1],
                             bias=lnbTs[:, c:c + 1])
